# revision 30
# baseline (speedup 1.0000x reference)
"""2-layer GRU (T=512, B=64, E=300, H=512) on 8 NeuronCores.

Strategy v8: 4-way batch-parallel x 2-way layer-pipeline, rebuilt per-step
critical path:
  - z-gate weights/biases NEGATED at prep, so zb = sigmoid(az) = 1-z;
    combine h' = (h - zb*h) + zb*n runs its (h - zb*h) half in parallel
    with tanh; final add is a fused scalar_tensor_tensor that also
    rescales (see below).
  - r/z recurrent weights in fp8 E3M4 at x64 scale, n-gate in bf16 x64;
    the state is stored as h/64 (bf16), compensated exactly by the fused
    v*(1/64)+u combine and a x64 on the host reduce. wib (layer-1 input
    proj) is also x64 since it consumes h1/64.
  - input projections computed TRANSPOSED (xpT[token,gate]) via a
    unified per-core input buffer xin = rxS*flagL1 + xT (layer-1 cores
    get zero embeddings so their gather contributes zero), 4 uniform
    k-matmuls per 512-gate bank; per-step xr/xzb enter PSUM via k=64
    selector matmuls (one per m-tile per 4-step quad, split r/z around
    the drip to avoid PE queue head-blocking).
  - n-gate hidden bias via one k=4 matmul FIRST (start=True); n-gate xp
    stays in gate-major layout for the an vector-add.
  - background work (projections, gathers, combine) dripped as ~1-matmul
    thunks between steps; no dummy warm matmuls.
Host: final 4-way partial-sum over layer-1 cores, x64/B, tiny FC.
"""
import numpy as np
import ml_dtypes

T, B, E, H, V, L = 512, 64, 300, 512, 30000, 5
NCORE = 8
DP = 4                    # batch-parallel width
BC = B // DP              # 16 sequences per core
CH = 32                   # steps per chunk/phase
NCH = T // CH             # 16 data chunks
NPH = NCH + 2             # 18 phases (layer-1 lags two chunks)
G3 = 3 * H                # 1536
G2 = 2 * H                # 1024 (r+z gate block, transposed proj)
KH = H // 128             # 4 k-tiles (contraction)
M3 = G3 // 128            # 12 m-tiles (output gates)
CB = CH * BC              # 512 cols per chunk
SEG = (NPH * CH + 1) * BC  # cols per k-segment of state
NBLK = T * BC // 128      # 64 gather blocks of 128 tokens
EK = 3                    # E=300 -> 3 partial k-tiles (128,128,44)
NSTEP = NPH * CH

_cache = {}


def _build():
    from contextlib import ExitStack
    import concourse.bass as bass
    import concourse.mybir as mybir
    import concourse.tile as tile
    from concourse import bacc
    from concourse.masks import make_identity

    bf16, f32, i32 = mybir.dt.bfloat16, mybir.dt.float32, mybir.dt.int32
    AF = mybir.ActivationFunctionType
    OP = mybir.AluOpType

    nc = bacc.Bacc("TRN2", target_bir_lowering=False, debug=False,
                   num_devices=NCORE)
    emb_d = nc.dram_tensor("emb", [V, E], f32, kind="ExternalInput").ap()
    idx_d = nc.dram_tensor("idx", [128, NBLK], i32, kind="ExternalInput").ap()
    whh8_d = nc.dram_tensor("whh8", [128, KH * G2], mybir.dt.float8e3,
                            kind="ExternalInput").ap()
    whhn_d = nc.dram_tensor("whhn", [128, KH * 512], bf16,
                            kind="ExternalInput").ap()
    wib_d = nc.dram_tensor("wib", [128, KH * G3], bf16, kind="ExternalInput").ap()
    fl1_d = nc.dram_tensor("fl1", [128, 1], f32, kind="ExternalInput").ap()
    # transposed-proj bias row (r,zb blocks, phase-gated) on partition 0
    brz_d = nc.dram_tensor("brz", [128, NPH * G2], bf16, kind="ExternalInput").ap()
    # n-gate input-proj bias (bih_n), per phase, for the ACT bias-copy
    b0n_d = nc.dram_tensor("b0n", [128, NPH * 4], f32, kind="ExternalInput").ap()
    # n-gate hidden bias wrapped k=4: bn4[j, p] = bhh_n[128j + p]
    bn4_d = nc.dram_tensor("bn4", [128, 128], bf16, kind="ExternalInput").ap()
    # phase-gated k=4 indicator: ind4[j, p*64 + 16mi + b] = act[p]*(mi == j)
    ind4_d = nc.dram_tensor("ind4", [128, NPH * 64], bf16, kind="ExternalInput").ap()
    # EYE64: sel[p, c] = (p % 64 == c), for the k=64 prefill matmuls
    sel_d = nc.dram_tensor("sel", [128, 64], bf16, kind="ExternalInput").ap()
    out_d = nc.dram_tensor("out", [128, KH * T], f32, kind="ExternalOutput").ap()
    # collective bounce buffers (double-buffered across phases)
    snd_d = [nc.dram_tensor(f"snd{i}", [128, KH * CB], bf16) for i in range(2)]
    rcv_d = [nc.dram_tensor(f"rcv{i}", [2, 128, KH * CB], bf16) for i in range(2)]
    GROUPS = [[c, c + DP] for c in range(DP)]

    with tile.TileContext(nc) as tc, ExitStack() as ctx:
        wp = ctx.enter_context(tc.tile_pool(name="wp", bufs=1))
        sp = ctx.enter_context(tc.tile_pool(name="sp", bufs=1))
        xb = ctx.enter_context(tc.tile_pool(name="xb", bufs=2))
        rxp = ctx.enter_context(tc.tile_pool(name="rxp", bufs=2))
        tp = ctx.enter_context(tc.tile_pool(name="tp", bufs=3))
        # PSUM: gate banks r, z, n(+tmp) = 3; projT 2; old-n-xp 2; transpose 1
        pg = ctx.enter_context(tc.tile_pool(name="pg", bufs=1, space="PSUM"))
        pj = ctx.enter_context(tc.tile_pool(name="pj", bufs=2, space="PSUM"))
        px = ctx.enter_context(tc.tile_pool(name="px", bufs=2, space="PSUM"))
        pt = ctx.enter_context(tc.tile_pool(name="pt", bufs=1, space="PSUM"))

        def wtile(nm, shape, dt, src=None):
            t = wp.tile(shape, dt, name=nm, tag=nm)
            if src is not None:
                nc.sync.dma_start(out=t[:], in_=src[:])
            return t

        whh8 = wtile("whh8_t", [128, KH * G2], mybir.dt.float8e3, whh8_d)
        whhn = wtile("whhn_t", [128, KH * 512], bf16, whhn_d)
        wib = wtile("wib_t", [128, KH * G3], bf16, wib_d)
        fl1 = wtile("fl1_t", [128, 1], f32, fl1_d)
        brz = wtile("brz_t", [128, NPH * G2], bf16, brz_d)
        b0n = wtile("b0n_t", [128, NPH * 4], f32, b0n_d)
        bn4 = wtile("bn4_t", [128, 128], bf16, bn4_d)
        ind4 = wtile("ind4_t", [128, NPH * 64], bf16, ind4_d)
        sel = wtile("sel_t", [128, 64], bf16, sel_d)
        idx_t = wtile("idx_t", [128, NBLK], i32, idx_d)
        ident = wtile("ident", [128, 128], bf16)
        make_identity(nc, ident[:])
        ones = wtile("ones", [128, 128], bf16)
        nc.vector.memset(ones[0:1, :], 1.0)

        st = sp.tile([128, KH * SEG], bf16, name="st", tag="st")
        pooled = sp.tile([128, KH * T], f32, name="pooled", tag="pooled")
        for k in range(KH):
            nc.vector.memset(st[:, k * SEG:k * SEG + BC], 0.0)
        rxS = []
        xin = []
        for i in range(2):
            r = sp.tile([128, KH * CB], bf16, name=f"rxS{i}", tag=f"rxS{i}")
            nc.vector.memset(r[:], 0.0)
            rxS.append(r)
            xi = sp.tile([128, KH * CB], bf16, name=f"xin{i}", tag=f"xin{i}")
            nc.vector.memset(xi[:], 0.0)
            xin.append(xi)
        # gate PSUM banks, persistent layout:
        #   prz_r: [0:64] even-step r, [64:128] odd-step r
        #   prz_z: same for zb;  pn: [0:64] gn, [64:128] hm, [128:192] an
        prz_r = pg.tile([128, 512], f32, name="przr", tag="przr")
        prz_z = pg.tile([128, 512], f32, name="przz", tag="przz")
        pn = pg.tile([128, 512], f32, name="pn", tag="pn")

        def gather_thunks(c, xT):
            """indirect-gather + transpose chunk c tokens into xT buffer"""
            out = []
            for j in range(4):
                box = {}

                def t_a(j=j, box=box):
                    blk = 4 * c + j
                    xg = tp.tile([128, E], f32, name="xg", tag="xg")
                    nc.gpsimd.indirect_dma_start(
                        out=xg[:], out_offset=None, in_=emb_d[:],
                        in_offset=bass.IndirectOffsetOnAxis(
                            ap=idx_t[:, blk:blk + 1], axis=0))
                    xc = tp.tile([128, E], bf16, name="xc", tag="xc")
                    nc.vector.tensor_copy(out=xc[:], in_=xg[:])
                    box["xc"] = xc

                def t_b(j=j, box=box):
                    xc = box["xc"]
                    for e in range(EK):
                        ke = min(128, E - e * 128)
                        tps = pt.tile([128, 128], bf16, name="tps", tag="tps")
                        nc.tensor.transpose(out=tps[0:ke, :],
                                            in_=xc[:, e * 128:e * 128 + ke],
                                            identity=ident[:])
                        nc.vector.tensor_copy(
                            out=xT[0:ke, e * CB + j * 128:e * CB + (j + 1) * 128],
                            in_=tps[0:ke, :])
                out.append((300, t_a))
                out.append((650, t_b))
            return out

        def projT_thunks(p, c, xi, xpT):
            """transposed input projection for phase p, chunk c, r+z gates,
            as (cost_ns, fn) thunks, one MM each."""
            out = []
            for g in range(2):           # gate block: 0=r, 1=zb
                box = {}

                def t_bias(g=g, box=box):
                    pp = pj.tile([128, 512], f32, name="ppj", tag="ppj")
                    box["pp"] = pp
                    nc.tensor.matmul(
                        out=pp[:, 0:512],
                        lhsT=ones[0:1, 0:128],
                        rhs=brz[0:1, p * G2 + g * 512:p * G2 + (g + 1) * 512],
                        start=True, stop=False)
                out.append((340, t_bias))

                for k in range(KH):
                    def t_k(g=g, box=box, k=k):
                        nc.tensor.matmul(
                            out=box["pp"][:, 0:512],
                            lhsT=xi[:, k * CB + c * 128:k * CB + (c + 1) * 128],
                            rhs=wib[:, k * G3 + g * 512:k * G3 + (g + 1) * 512],
                            start=False, stop=(k == KH - 1))
                    out.append((430, t_k))

                def t_c1(g=g, box=box):
                    nc.scalar.activation(
                        out=xpT[:, c * G2 + g * 512:c * G2 + g * 512 + 256],
                        in_=box["pp"][:, 0:256], func=AF.Identity)

                def t_c2(g=g, box=box):
                    nc.scalar.activation(
                        out=xpT[:, c * G2 + g * 512 + 256:c * G2 + (g + 1) * 512],
                        in_=box["pp"][:, 256:512], func=AF.Identity)
                out.append((200, t_c1))
                out.append((200, t_c2))
            return out

        def xpn_thunks(p, i, xi, xpb):
            """input projection for the n-gate m-tile i (of 4)."""
            m = 8 + i
            box = {}
            out = []
            for k in range(KH):
                def t_k(k=k):
                    if k == 0:
                        box["pp"] = px.tile([128, CB], f32, name="xpp", tag="xpp")
                    nc.tensor.matmul(
                        out=box["pp"][:, 0:CB],
                        lhsT=wib[:, k * G3 + m * 128:k * G3 + (m + 1) * 128],
                        rhs=xi[:, k * CB:(k + 1) * CB],
                        start=(k == 0), stop=(k == KH - 1))
                out.append((430, t_k))

            def t_c1():
                nc.vector.tensor_scalar_add(
                    out=xpb[:, i * CB:i * CB + 256], in0=box["pp"][:, 0:256],
                    scalar1=b0n[:, p * 4 + i:p * 4 + i + 1])

            def t_c2():
                nc.vector.tensor_scalar_add(
                    out=xpb[:, i * CB + 256:(i + 1) * CB],
                    in0=box["pp"][:, 256:512],
                    scalar1=b0n[:, p * 4 + i:p * 4 + i + 1])
            out.append((200, t_c1))
            out.append((200, t_c2))
            return out

        def prefill(t, xpT, gate, pin=None):
            """pre-add xr (gate=0) or xzb (gate=1) for steps (t..t+3) into
            the bank col-sets. One k=64 selector matmul per m-tile."""
            from contextlib import nullcontext
            ts = t % CH
            cc = ts // 8
            base = 16 * (ts % 8)        # 0 or 64 because t % 4 == 0
            bank = prz_r if gate == 0 else prz_z
            with tc.tile_wait_until(pin) if pin is not None else nullcontext():
                bv = bank[:, 0:256].rearrange("p (q c) -> p q c", q=4)
                for mi in range(4):
                    nc.tensor.matmul(
                        out=bv[:, :, 16 * mi:16 * mi + 16],
                        lhsT=xpT[base:base + 64,
                                 cc * G2 + gate * 512 + 128 * mi:
                                 cc * G2 + gate * 512 + 128 * (mi + 1)],
                        rhs=sel[base:base + 64, :]
                            .rearrange("p (q c) -> p q c", q=4),
                        start=True, stop=False, skip_group_check=True)

        def step(p, ts, xpb):
            t = p * CH + ts
            hc = 64 * (ts % 4)          # r/z bank col-set for this step
            pin = (10 * t) / 10000.0

            def mm_gates(bank, gate):
                for mi in range(4):
                    m = gate * 4 + mi
                    for k in range(KH):
                        nc.tensor.matmul(
                            out=bank[:, hc + mi * BC:hc + (mi + 1) * BC],
                            lhsT=whh8[:, k * G2 + m * 128:k * G2 + (m + 1) * 128],
                            rhs=st[:, k * SEG + t * BC:k * SEG + (t + 1) * BC],
                            start=False, stop=(k == KH - 1),
                            skip_group_check=True)

            # bias-MM first (fills the h'-wait), then r, n, z groups
            nc.tensor.matmul(
                out=pn[:, 0:64],
                lhsT=bn4[0:4, 0:128],
                rhs=ind4[0:4, p * 64:(p + 1) * 64],
                start=True, stop=False, skip_group_check=True)
            mm_gates(prz_r, 0)
            for mi in range(4):
                for k in range(KH):
                    nc.tensor.matmul(
                        out=pn[:, mi * BC:(mi + 1) * BC],
                        lhsT=whhn[:, k * 512 + mi * 128:k * 512 + (mi + 1) * 128],
                        rhs=st[:, k * SEG + t * BC:k * SEG + (t + 1) * BC],
                        start=False, stop=(k == KH - 1),
                        skip_group_check=True)
            mm_gates(prz_z, 1)

            def tmp(nm, dt=f32):
                return tp.tile([128, KH * BC], dt, name=nm, tag=nm)

            # ACT queue: sig(r), sig(zb), tanh -- in this order
            r = tmp("r", bf16)
            with tc.tile_wait_until(pin + 1 / 10000.0):
                nc.scalar.activation(out=r[:], in_=prz_r[:, hc:hc + 64],
                                     func=AF.Sigmoid)
            zb = tmp("zb", bf16)
            with tc.tile_wait_until(pin + 3 / 10000.0):
                nc.scalar.activation(out=zb[:], in_=prz_z[:, hc:hc + 64],
                                     func=AF.Sigmoid)

            # DVE queue: hm, an, t1, u, v, h'
            with tc.tile_wait_until(pin + 2 / 10000.0):
                nc.vector.tensor_tensor(out=pn[:, 64:128], in0=pn[:, 0:64],
                                        in1=r[:], op=OP.mult)
            xpv = xpb[:].rearrange("p (m s) -> p m s", m=4)
            sl = ts * BC
            with tc.tile_wait_until(pin + 2.4 / 10000.0):
                nc.vector.tensor_tensor(
                    out=pn[:, 128:192].rearrange("p (m b) -> p m b", b=BC),
                    in0=xpv[:, :, sl:sl + BC],
                    in1=pn[:, 64:128].rearrange("p (m b) -> p m b", b=BC),
                    op=OP.add)
            n = tmp("n", bf16)
            with tc.tile_wait_until(pin + 4 / 10000.0):
                nc.scalar.activation(out=n[:], in_=pn[:, 128:192], func=AF.Tanh)

            stv = st[:].rearrange("p (c s) -> p c s", c=KH)
            hview = stv[:, :, t * BC:(t + 1) * BC]
            t1 = tmp("t1")
            with tc.tile_wait_until(pin + 4.4 / 10000.0):
                nc.vector.tensor_tensor(
                    out=t1[:].rearrange("p (c b) -> p c b", c=KH),
                    in0=hview, in1=zb[:].rearrange("p (c b) -> p c b", c=KH),
                    op=OP.mult)
            u = tmp("u")
            nc.vector.tensor_tensor(
                out=u[:].rearrange("p (c b) -> p c b", c=KH),
                in0=hview,
                in1=t1[:].rearrange("p (c b) -> p c b", c=KH),
                op=OP.subtract)
            v = tmp("v")
            with tc.tile_wait_until(pin + 6 / 10000.0):
                nc.vector.tensor_tensor(out=v[:], in0=zb[:], in1=n[:],
                                        op=OP.mult)
            nc.vector.scalar_tensor_tensor(
                out=stv[:, :, (t + 1) * BC:(t + 2) * BC],
                in0=v[:].rearrange("p (c b) -> p c b", c=KH),
                scalar=1.0 / 64.0,
                in1=u[:].rearrange("p (c b) -> p c b", c=KH),
                op0=OP.mult, op1=OP.add)

        def reduce_thunks(q):
            """partial batch-sum of phase q's states into pooled (4 pieces)"""
            out = []
            for piece in range(4):
                def t_r(piece=piece):
                    ts0 = piece * 8
                    nc.vector.tensor_reduce(
                        out=pooled[:].rearrange("p (c t) -> p c t", c=KH)
                            [:, :, (q - 2) * CH + ts0:(q - 2) * CH + ts0 + 8],
                        in_=st[:].rearrange("p (c s b) -> p c s b", c=KH, b=BC)
                            [:, :, q * CH + 1 + ts0:q * CH + 1 + ts0 + 8, :],
                        axis=mybir.AxisListType.X, op=OP.add)
                out.append((600, t_r))
            return out

        # ---- main schedule ----
        xTb = []
        for i in range(2):
            xt = xb.tile([128, KH * CB], bf16, name=f"xT{i}", tag=f"xT{i}")
            nc.vector.memset(xt[:], 0.0)
            xTb.append(xt)
        xpTb = []
        for i in range(2):
            xpTb.append(sp.tile([128, 4 * G2], bf16, name=f"xpT{i}",
                                tag=f"xpT{i}"))
        for _, th in gather_thunks(0, xTb[0]):
            th()
        for _, th in gather_thunks(1, xTb[1]):
            th()
        xpb_cur = rxp.tile([128, 4 * CB], bf16, name="xpb", tag="xpb")

        def combine(i, q):
            nc.vector.scalar_tensor_tensor(
                out=xin[i][:, q * 256:(q + 1) * 256],
                in0=rxS[i][:, q * 256:(q + 1) * 256], scalar=fl1[:, 0:1],
                in1=xTb[i][:, q * 256:(q + 1) * 256], op0=OP.mult, op1=OP.add)

        for q in range(8):
            combine(0, q)
        for c in range(4):
            for _, th in projT_thunks(0, c, xin[0], xpTb[0]):
                th()
        for i in range(4):
            for _, th in xpn_thunks(0, i, xin[0], xpb_cur):
                th()
        prefill(0, xpTb[0], 0)
        prefill(0, xpTb[0], 1)
        for p in range(NPH):
            xpb_next = (rxp.tile([128, 4 * CB], bf16, name="xpb", tag="xpb")
                        if p + 1 < NPH else None)
            xpT_next = xpTb[(p + 1) % 2]
            # background work queue for this phase, dripped ~2 thunks/step
            work = []
            if p + 1 < NPH:
                i2 = (p + 1) % 2
                for q in range(8):
                    work.append((420, lambda i2=i2, q=q: combine(i2, q)))
                for c in range(4):
                    work += projT_thunks(p + 1, c, xin[i2], xpT_next)
                    work += xpn_thunks(p + 1, c, xin[i2], xpb_next)
            if p + 2 < NCH:
                work += gather_thunks(p + 2, xTb[(p + 2) % 2])
            if 2 <= p - 1:
                work += reduce_thunks(p - 1)
            wi = 0
            spent = 0
            nwk = len(work)
            for ts in range(CH):
                t = p * CH + ts
                pin = (10 * t + 7) / 10000.0
                step(p, ts, xpb_cur)
                # prefill for the next step quad: r-part first (waits
                # sig-r only), drip, then z-part (waits sig-zb)
                pf = t + 1 < NSTEP and ts % 4 == 3
                if pf:
                    t2 = t + 1
                    prefill(t2, xpTb[(t2 // CH) % 2], 0, pin=pin)
                # drip this phase's background work into the chain gaps
                if ts >= 6 and wi < nwk:
                    cap = (ts - 5) * 1000
                    if spent < cap:
                        with tc.tile_wait_until(pin):
                            while wi < nwk and spent < cap:
                                spent += work[wi][0]
                                work[wi][1]()
                                wi += 1
                if pf:
                    prefill(t2, xpTb[(t2 // CH) % 2], 1, pin=pin)
            # flush any un-dripped background work before the phase ends
            while wi < len(work):
                work[wi][1]()
                wi += 1
            if p < NCH:
                io = p % 2
                stv = st[:].rearrange("p (c s) -> p c s", c=KH)
                nc.gpsimd.dma_start(
                    out=snd_d[io].ap(),
                    in_=stv[:, :, (p * CH + 1) * BC:(p * CH + 1 + CH) * BC])
                nc.gpsimd.collective_compute(
                    "AllGather", mybir.AluOpType.bypass,
                    replica_groups=GROUPS,
                    ins=[snd_d[io].ap().opt()],
                    outs=[rcv_d[io].ap().opt()])
                nc.gpsimd.dma_start(out=rxS[p % 2][:],
                                    in_=rcv_d[io].ap()[0])
            xpb_cur = xpb_next

        for _, th in reduce_thunks(NPH - 1):
            th()
        nc.sync.dma_start(out=out_d[:], in_=pooled[:])

    nc.compile()
    return nc


def _prep(inputs):
    bf = ml_dtypes.bfloat16

    def packT(W, nk, negate_z=False, scale=1.0, rows=None, dt=None):
        Wf = np.asarray(W, np.float32).copy()
        if negate_z:
            Wf[H:2 * H] = -Wf[H:2 * H]
        Wf *= scale
        if rows is not None:
            Wf = Wf[rows[0]:rows[1]]
        WT = np.ascontiguousarray(Wf.T)
        K, G = WT.shape
        pad = np.zeros((nk * 128, G), np.float32)
        pad[:K] = WT
        return np.concatenate([pad[k * 128:(k + 1) * 128] for k in range(nk)],
                              axis=1).astype(dt if dt is not None else bf)

    texts = np.asarray(inputs["texts"])
    zidx = np.zeros((128, NBLK), np.int32)
    zemb = np.zeros((V, E), np.float32)
    sel = np.zeros((128, 64), bf)
    for pp in range(128):
        sel[pp, pp % 64] = 1.0
    in_maps = []
    for c in range(NCORE):
        l0 = c < DP
        q = c % DP
        act = [p < NCH for p in range(NPH)] if l0 else \
              [2 <= p for p in range(NPH)]
        Wih, Whh = (inputs["Wih0"], inputs["Whh0"]) if l0 else \
                   (inputs["Wih1"], inputs["Whh1"])
        bih, bhh = (inputs["bih0"], inputs["bhh0"]) if l0 else \
                   (inputs["bih1"], inputs["bhh1"])
        bih = np.asarray(bih, np.float32)
        bhh = np.asarray(bhh, np.float32)
        # transposed-proj bias row: r block, then NEGATED z block
        brow = np.zeros(G2, np.float32)
        brow[0:H] = bih[0:H] + bhh[0:H]
        brow[H:2 * H] = -(bih[H:2 * H] + bhh[H:2 * H])
        brz = np.zeros((128, NPH * G2), np.float32)
        b0n = np.zeros((128, NPH * 4), np.float32)
        ind4 = np.zeros((128, NPH * 64), np.float32)
        for p in range(NPH):
            if act[p]:
                brz[0, p * G2:(p + 1) * G2] = brow
                for mi in range(4):
                    b0n[:, p * 4 + mi] = bih[2 * H + 128 * mi:
                                             2 * H + 128 * (mi + 1)]
                    ind4[mi, p * 64 + 16 * mi:p * 64 + 16 * (mi + 1)] = 1.0
        bn4 = np.zeros((128, 128), np.float32)
        for j in range(4):
            bn4[j, :] = bhh[2 * H + 128 * j:2 * H + 128 * (j + 1)]
        idxc = np.ascontiguousarray(
            texts[:, q * BC:(q + 1) * BC].astype(np.int32)
            .reshape(NBLK, 128).T) if l0 else zidx
        in_maps.append({
            "emb": (np.ascontiguousarray(inputs["emb"], dtype=np.float32)
                    if l0 else zemb),
            "idx": idxc,
            "whh8": packT(Whh, KH, negate_z=True, scale=64.0, rows=(0, G2),
                          dt=ml_dtypes.float8_e3m4),
            "whhn": packT(Whh, KH, scale=64.0, rows=(G2, G3)),
            "wib": packT(Wih, KH, negate_z=True,
                         scale=(1.0 if l0 else 64.0)),
            "fl1": np.full((128, 1), 0.0 if l0 else 1.0, np.float32),
            "brz": brz.astype(bf),
            "b0n": b0n,
            "bn4": bn4.astype(bf),
            "ind4": ind4.astype(bf),
            "sel": sel,
        })
    return in_maps


def _postproc(results):
    s = np.zeros((128, KH * T), np.float32)
    for c in range(DP, NCORE):
        s += results[c]["out"]
    return s.reshape(128, KH, T).transpose(2, 1, 0).reshape(T, H) * (64.0 / B)


def kernel(**inputs):
    from concourse import bass_utils
    if "nc" not in _cache:
        _cache["nc"] = _build()
    nc = _cache["nc"]
    in_maps = _prep(inputs)
    res = bass_utils.run_bass_kernel_spmd(
        nc, in_maps, core_ids=list(range(NCORE)))
    pooled = _postproc([res.results[i] for i in range(NCORE)])
    fc_W = np.asarray(inputs["fc_W"], dtype=np.float32)
    fc_b = np.asarray(inputs["fc_b"], dtype=np.float32)
    return (pooled @ fc_W.T + fc_b).astype(np.float32)


if __name__ == "__main__":
    import time
    t0 = time.time()
    nc = _build()
    print("build+compile time:", round(time.time() - t0, 1), "s")


# revision 31
# speedup vs baseline: 1.2068x; 1.2068x over previous
"""2-layer GRU (T=512, B=64, E=300, H=512) on 8 NeuronCores.

Strategy v8: 4-way batch-parallel x 2-way layer-pipeline, rebuilt per-step
critical path:
  - z-gate weights/biases NEGATED at prep, so zb = sigmoid(az) = 1-z;
    combine h' = (h - zb*h) + zb*n runs its (h - zb*h) half in parallel
    with tanh; final add is a fused scalar_tensor_tensor that also
    rescales (see below).
  - r/z recurrent weights in fp8 E3M4 at x64 scale, n-gate in bf16 x64;
    the state is stored as h/64 (bf16), compensated exactly by the fused
    v*(1/64)+u combine and a x64 on the host reduce. wib (layer-1 input
    proj) is also x64 since it consumes h1/64.
  - input projections computed TRANSPOSED (xpT[token,gate]) via a
    unified per-core input buffer xin = rxS*flagL1 + xT (layer-1 cores
    get zero embeddings so their gather contributes zero), 4 uniform
    k-matmuls per 512-gate bank; per-step xr/xzb enter PSUM via k=64
    selector matmuls (one per m-tile per 4-step quad, split r/z around
    the drip to avoid PE queue head-blocking).
  - n-gate hidden bias via one k=4 matmul FIRST (start=True); n-gate xp
    stays in gate-major layout for the an vector-add.
  - background work (projections, gathers, combine) dripped as ~1-matmul
    thunks between steps; no dummy warm matmuls.
Host: final 4-way partial-sum over layer-1 cores, x64/B, tiny FC.
"""
import numpy as np
import ml_dtypes

T, B, E, H, V, L = 512, 64, 300, 512, 30000, 5
NCORE = 8
DP = 4                    # batch-parallel width
BC = B // DP              # 16 sequences per core
CH = 32                   # steps per chunk/phase
NCH = T // CH             # 16 data chunks
NPH = NCH + 2             # 18 phases (layer-1 lags two chunks)
G3 = 3 * H                # 1536
G2 = 2 * H                # 1024 (r+z gate block, transposed proj)
KH = H // 128             # 4 k-tiles (contraction)
M3 = G3 // 128            # 12 m-tiles (output gates)
CB = CH * BC              # 512 cols per chunk
SEG = (NPH * CH + 1) * BC  # cols per k-segment of state
NBLK = T * BC // 128      # 64 gather blocks of 128 tokens
EK = 3                    # E=300 -> 3 partial k-tiles (128,128,44)
NSTEP = NPH * CH

_cache = {}


def _build():
    from contextlib import ExitStack
    import concourse.bass as bass
    import concourse.mybir as mybir
    import concourse.tile as tile
    from concourse import bacc
    from concourse.masks import make_identity

    bf16, f32, i32 = mybir.dt.bfloat16, mybir.dt.float32, mybir.dt.int32
    AF = mybir.ActivationFunctionType
    OP = mybir.AluOpType

    nc = bacc.Bacc("TRN2", target_bir_lowering=False, debug=False,
                   num_devices=NCORE)
    emb_d = nc.dram_tensor("emb", [V, E], f32, kind="ExternalInput").ap()
    idx_d = nc.dram_tensor("idx", [128, NBLK], i32, kind="ExternalInput").ap()
    whh8_d = nc.dram_tensor("whh8", [128, KH * G2], mybir.dt.float8e3,
                            kind="ExternalInput").ap()
    whhn_d = nc.dram_tensor("whhn", [128, KH * 512], bf16,
                            kind="ExternalInput").ap()
    wib_d = nc.dram_tensor("wib", [128, KH * G3], bf16, kind="ExternalInput").ap()
    fl1_d = nc.dram_tensor("fl1", [128, 1], f32, kind="ExternalInput").ap()
    # transposed-proj bias row (r,zb blocks, phase-gated) on partition 0
    brz_d = nc.dram_tensor("brz", [128, NPH * G2], bf16, kind="ExternalInput").ap()
    # n-gate input-proj bias (bih_n), per phase, for the ACT bias-copy
    b0n_d = nc.dram_tensor("b0n", [128, NPH * 4], f32, kind="ExternalInput").ap()
    # n-gate hidden bias wrapped k=4: bn4[j, p] = bhh_n[128j + p]
    bn4_d = nc.dram_tensor("bn4", [128, 128], bf16, kind="ExternalInput").ap()
    # phase-gated k=4 indicator: ind4[j, p*64 + 16mi + b] = act[p]*(mi == j)
    ind4_d = nc.dram_tensor("ind4", [128, NPH * 64], bf16, kind="ExternalInput").ap()
    # EYE64: sel[p, c] = (p % 64 == c), for the k=64 prefill matmuls
    sel_d = nc.dram_tensor("sel", [128, 64], bf16, kind="ExternalInput").ap()
    out_d = nc.dram_tensor("out", [128, KH * T], f32, kind="ExternalOutput").ap()
    # collective bounce buffers (double-buffered across phases)
    snd_d = [nc.dram_tensor(f"snd{i}", [128, KH * CB], bf16) for i in range(2)]
    rcv_d = [nc.dram_tensor(f"rcv{i}", [2, 128, KH * CB], bf16) for i in range(2)]
    GROUPS = [[c, c + DP] for c in range(DP)]

    with tile.TileContext(nc) as tc, ExitStack() as ctx:
        wp = ctx.enter_context(tc.tile_pool(name="wp", bufs=1))
        sp = ctx.enter_context(tc.tile_pool(name="sp", bufs=1))
        xb = ctx.enter_context(tc.tile_pool(name="xb", bufs=2))
        rxp = ctx.enter_context(tc.tile_pool(name="rxp", bufs=2))
        tp = ctx.enter_context(tc.tile_pool(name="tp", bufs=3))
        # PSUM: gate banks r, z, n(+tmp) = 3; projT 2; old-n-xp 2; transpose 1
        pg = ctx.enter_context(tc.tile_pool(name="pg", bufs=1, space="PSUM"))
        pj = ctx.enter_context(tc.tile_pool(name="pj", bufs=2, space="PSUM"))
        px = ctx.enter_context(tc.tile_pool(name="px", bufs=2, space="PSUM"))
        pt = ctx.enter_context(tc.tile_pool(name="pt", bufs=1, space="PSUM"))

        def wtile(nm, shape, dt, src=None):
            t = wp.tile(shape, dt, name=nm, tag=nm)
            if src is not None:
                nc.sync.dma_start(out=t[:], in_=src[:])
            return t

        whh8 = wtile("whh8_t", [128, KH * G2], mybir.dt.float8e3, whh8_d)
        whhn = wtile("whhn_t", [128, KH * 512], bf16, whhn_d)
        wib = wtile("wib_t", [128, KH * G3], bf16, wib_d)
        fl1 = wtile("fl1_t", [128, 1], f32, fl1_d)
        brz = wtile("brz_t", [128, NPH * G2], bf16, brz_d)
        b0n = wtile("b0n_t", [128, NPH * 4], f32, b0n_d)
        bn4 = wtile("bn4_t", [128, 128], bf16, bn4_d)
        ind4 = wtile("ind4_t", [128, NPH * 64], bf16, ind4_d)
        sel = wtile("sel_t", [128, 64], bf16, sel_d)
        idx_t = wtile("idx_t", [128, NBLK], i32, idx_d)
        ident = wtile("ident", [128, 128], bf16)
        make_identity(nc, ident[:])
        ones = wtile("ones", [128, 128], bf16)
        nc.vector.memset(ones[0:1, :], 1.0)

        st = sp.tile([128, KH * SEG], bf16, name="st", tag="st")
        pooled = sp.tile([128, KH * T], f32, name="pooled", tag="pooled")
        for k in range(KH):
            nc.vector.memset(st[:, k * SEG:k * SEG + BC], 0.0)
        rxS = []
        xin = []
        for i in range(2):
            r = sp.tile([128, KH * CB], bf16, name=f"rxS{i}", tag=f"rxS{i}")
            nc.vector.memset(r[:], 0.0)
            rxS.append(r)
            xi = sp.tile([128, KH * CB], bf16, name=f"xin{i}", tag=f"xin{i}")
            nc.vector.memset(xi[:], 0.0)
            xin.append(xi)
        # gate PSUM banks, persistent layout:
        #   prz_r: [0:64] even-step r, [64:128] odd-step r
        #   prz_z: same for zb;  pn: [0:64] gn, [64:128] hm, [128:192] an
        prz_r = pg.tile([128, 512], f32, name="przr", tag="przr")
        prz_z = pg.tile([128, 512], f32, name="przz", tag="przz")
        pn = pg.tile([128, 512], f32, name="pn", tag="pn")

        def gather_thunks(c, xT):
            """indirect-gather + transpose chunk c tokens into xT buffer"""
            out = []
            for j in range(4):
                box = {}

                def t_a(j=j, box=box):
                    blk = 4 * c + j
                    xg = tp.tile([128, E], f32, name="xg", tag="xg")
                    nc.gpsimd.indirect_dma_start(
                        out=xg[:], out_offset=None, in_=emb_d[:],
                        in_offset=bass.IndirectOffsetOnAxis(
                            ap=idx_t[:, blk:blk + 1], axis=0))
                    xc = tp.tile([128, E], bf16, name="xc", tag="xc")
                    nc.vector.tensor_copy(out=xc[:], in_=xg[:])
                    box["xc"] = xc

                def t_b(j=j, box=box):
                    xc = box["xc"]
                    for e in range(EK):
                        ke = min(128, E - e * 128)
                        tps = pt.tile([128, 128], bf16, name="tps", tag="tps")
                        nc.tensor.transpose(out=tps[0:ke, :],
                                            in_=xc[:, e * 128:e * 128 + ke],
                                            identity=ident[:])
                        nc.vector.tensor_copy(
                            out=xT[0:ke, e * CB + j * 128:e * CB + (j + 1) * 128],
                            in_=tps[0:ke, :])
                out.append((300, t_a))
                out.append((650, t_b))
            return out

        def projT_thunks(p, c, xi, xpT):
            """transposed input projection for phase p, chunk c, r+z gates,
            as (cost_ns, fn) thunks, one MM each."""
            out = []
            for g in range(2):           # gate block: 0=r, 1=zb
                box = {}

                def t_bias(g=g, box=box):
                    pp = pj.tile([128, 512], f32, name="ppj", tag="ppj")
                    box["pp"] = pp
                    nc.tensor.matmul(
                        out=pp[:, 0:512],
                        lhsT=ones[0:1, 0:128],
                        rhs=brz[0:1, p * G2 + g * 512:p * G2 + (g + 1) * 512],
                        start=True, stop=False)
                out.append((340, t_bias))

                for k in range(KH):
                    def t_k(g=g, box=box, k=k):
                        nc.tensor.matmul(
                            out=box["pp"][:, 0:512],
                            lhsT=xi[:, k * CB + c * 128:k * CB + (c + 1) * 128],
                            rhs=wib[:, k * G3 + g * 512:k * G3 + (g + 1) * 512],
                            start=False, stop=(k == KH - 1))
                    out.append((430, t_k))

                def t_c1(g=g, box=box):
                    nc.scalar.activation(
                        out=xpT[:, c * G2 + g * 512:c * G2 + g * 512 + 256],
                        in_=box["pp"][:, 0:256], func=AF.Identity)

                def t_c2(g=g, box=box):
                    nc.scalar.activation(
                        out=xpT[:, c * G2 + g * 512 + 256:c * G2 + (g + 1) * 512],
                        in_=box["pp"][:, 256:512], func=AF.Identity)
                out.append((200, t_c1))
                out.append((200, t_c2))
            return out

        def xpn_thunks(p, i, xi, xpb):
            """input projection for the n-gate m-tile i (of 4)."""
            m = 8 + i
            box = {}
            out = []
            for k in range(KH):
                def t_k(k=k):
                    if k == 0:
                        box["pp"] = px.tile([128, CB], f32, name="xpp", tag="xpp")
                    nc.tensor.matmul(
                        out=box["pp"][:, 0:CB],
                        lhsT=wib[:, k * G3 + m * 128:k * G3 + (m + 1) * 128],
                        rhs=xi[:, k * CB:(k + 1) * CB],
                        start=(k == 0), stop=(k == KH - 1))
                out.append((430, t_k))

            def t_c1():
                nc.vector.tensor_scalar_add(
                    out=xpb[:, i * CB:i * CB + 256], in0=box["pp"][:, 0:256],
                    scalar1=b0n[:, p * 4 + i:p * 4 + i + 1])

            def t_c2():
                nc.vector.tensor_scalar_add(
                    out=xpb[:, i * CB + 256:(i + 1) * CB],
                    in0=box["pp"][:, 256:512],
                    scalar1=b0n[:, p * 4 + i:p * 4 + i + 1])
            out.append((200, t_c1))
            out.append((200, t_c2))
            return out

        def prefill(t, xpT, gate, pin=None):
            """pre-add xr (gate=0) or xzb (gate=1) for steps (t..t+3) into
            the bank col-sets. One k=64 selector matmul per m-tile."""
            from contextlib import nullcontext
            ts = t % CH
            cc = ts // 8
            base = 16 * (ts % 8)        # 0 or 64 because t % 4 == 0
            bank = prz_r if gate == 0 else prz_z
            with tc.tile_wait_until(pin) if pin is not None else nullcontext():
                bv = bank[:, 0:256].rearrange("p (q c) -> p q c", q=4)
                for mi in range(4):
                    nc.tensor.matmul(
                        out=bv[:, :, 16 * mi:16 * mi + 16],
                        lhsT=xpT[base:base + 64,
                                 cc * G2 + gate * 512 + 128 * mi:
                                 cc * G2 + gate * 512 + 128 * (mi + 1)],
                        rhs=sel[base:base + 64, :]
                            .rearrange("p (q c) -> p q c", q=4),
                        start=True, stop=False, skip_group_check=True)

        def step(p, ts, xpb):
            t = p * CH + ts
            hc = 64 * (ts % 4)          # r/z bank col-set for this step
            pin = (10 * t) / 10000.0

            def mm_gates(bank, gate):
                for mi in range(4):
                    m = gate * 4 + mi
                    for k in range(KH):
                        nc.tensor.matmul(
                            out=bank[:, hc + mi * BC:hc + (mi + 1) * BC],
                            lhsT=whh8[:, k * G2 + m * 128:k * G2 + (m + 1) * 128],
                            rhs=st[:, k * SEG + t * BC:k * SEG + (t + 1) * BC],
                            start=False, stop=(k == KH - 1),
                            skip_group_check=True)

            # bias-MM first (fills the h'-wait), then r, n, z groups
            nc.tensor.matmul(
                out=pn[:, 0:64],
                lhsT=bn4[0:4, 0:128],
                rhs=ind4[0:4, p * 64:(p + 1) * 64],
                start=True, stop=False, skip_group_check=True)
            mm_gates(prz_r, 0)
            for mi in range(4):
                for k in range(KH):
                    nc.tensor.matmul(
                        out=pn[:, mi * BC:(mi + 1) * BC],
                        lhsT=whhn[:, k * 512 + mi * 128:k * 512 + (mi + 1) * 128],
                        rhs=st[:, k * SEG + t * BC:k * SEG + (t + 1) * BC],
                        start=False, stop=(k == KH - 1),
                        skip_group_check=True)
            mm_gates(prz_z, 1)

            def tmp(nm, dt=f32):
                return tp.tile([128, KH * BC], dt, name=nm, tag=nm)

            # ACT queue: sig(r), sig(zb), tanh -- in this order
            r = tmp("r", bf16)
            with tc.tile_wait_until(pin + 1 / 10000.0):
                nc.scalar.activation(out=r[:], in_=prz_r[:, hc:hc + 64],
                                     func=AF.Sigmoid)
            zb = tmp("zb", bf16)
            with tc.tile_wait_until(pin + 3 / 10000.0):
                nc.scalar.activation(out=zb[:], in_=prz_z[:, hc:hc + 64],
                                     func=AF.Sigmoid)

            # DVE queue: hm, an, t1, u, v, h'
            with tc.tile_wait_until(pin + 2 / 10000.0):
                nc.vector.tensor_tensor(out=pn[:, 64:128], in0=pn[:, 0:64],
                                        in1=r[:], op=OP.mult)
            xpv = xpb[:].rearrange("p (m s) -> p m s", m=4)
            sl = ts * BC
            with tc.tile_wait_until(pin + 2.4 / 10000.0):
                nc.vector.tensor_tensor(
                    out=pn[:, 128:192].rearrange("p (m b) -> p m b", b=BC),
                    in0=xpv[:, :, sl:sl + BC],
                    in1=pn[:, 64:128].rearrange("p (m b) -> p m b", b=BC),
                    op=OP.add)
            n = tmp("n", bf16)
            with tc.tile_wait_until(pin + 4 / 10000.0):
                nc.scalar.activation(out=n[:], in_=pn[:, 128:192], func=AF.Tanh)

            stv = st[:].rearrange("p (c s) -> p c s", c=KH)
            hview = stv[:, :, t * BC:(t + 1) * BC]
            t1 = tmp("t1")
            with tc.tile_wait_until(pin + 4.4 / 10000.0):
                nc.vector.tensor_tensor(
                    out=t1[:].rearrange("p (c b) -> p c b", c=KH),
                    in0=hview, in1=zb[:].rearrange("p (c b) -> p c b", c=KH),
                    op=OP.mult)
            u = tmp("u")
            nc.vector.tensor_tensor(
                out=u[:].rearrange("p (c b) -> p c b", c=KH),
                in0=hview,
                in1=t1[:].rearrange("p (c b) -> p c b", c=KH),
                op=OP.subtract)
            v = tmp("v")
            with tc.tile_wait_until(pin + 6 / 10000.0):
                nc.vector.tensor_tensor(out=v[:], in0=zb[:], in1=n[:],
                                        op=OP.mult)
            nc.vector.scalar_tensor_tensor(
                out=stv[:, :, (t + 1) * BC:(t + 2) * BC],
                in0=v[:].rearrange("p (c b) -> p c b", c=KH),
                scalar=1.0 / 64.0,
                in1=u[:].rearrange("p (c b) -> p c b", c=KH),
                op0=OP.mult, op1=OP.add)

        def reduce_thunks(q):
            """partial batch-sum of phase q's states into pooled (4 pieces)"""
            out = []
            for piece in range(4):
                def t_r(piece=piece):
                    ts0 = piece * 8
                    nc.vector.tensor_reduce(
                        out=pooled[:].rearrange("p (c t) -> p c t", c=KH)
                            [:, :, (q - 2) * CH + ts0:(q - 2) * CH + ts0 + 8],
                        in_=st[:].rearrange("p (c s b) -> p c s b", c=KH, b=BC)
                            [:, :, q * CH + 1 + ts0:q * CH + 1 + ts0 + 8, :],
                        axis=mybir.AxisListType.X, op=OP.add)
                out.append((600, t_r))
            return out

        # ---- main schedule ----
        xTb = []
        for i in range(2):
            xt = xb.tile([128, KH * CB], bf16, name=f"xT{i}", tag=f"xT{i}")
            nc.vector.memset(xt[:], 0.0)
            xTb.append(xt)
        xpTb = []
        for i in range(2):
            xpTb.append(sp.tile([128, 4 * G2], bf16, name=f"xpT{i}",
                                tag=f"xpT{i}"))
        for _, th in gather_thunks(0, xTb[0]):
            th()
        for _, th in gather_thunks(1, xTb[1]):
            th()
        xpb_cur = rxp.tile([128, 4 * CB], bf16, name="xpb", tag="xpb")

        def combine(i, q):
            nc.vector.scalar_tensor_tensor(
                out=xin[i][:, q * 256:(q + 1) * 256],
                in0=rxS[i][:, q * 256:(q + 1) * 256], scalar=fl1[:, 0:1],
                in1=xTb[i][:, q * 256:(q + 1) * 256], op0=OP.mult, op1=OP.add)

        for q in range(8):
            combine(0, q)
        for c in range(4):
            for _, th in projT_thunks(0, c, xin[0], xpTb[0]):
                th()
        for i in range(4):
            for _, th in xpn_thunks(0, i, xin[0], xpb_cur):
                th()
        prefill(0, xpTb[0], 0)
        prefill(0, xpTb[0], 1)
        for p in range(NPH):
            xpb_next = (rxp.tile([128, 4 * CB], bf16, name="xpb", tag="xpb")
                        if p + 1 < NPH else None)
            xpT_next = xpTb[(p + 1) % 2]
            # background work queue for this phase, dripped ~2 thunks/step
            work = []
            if p + 1 < NPH:
                i2 = (p + 1) % 2
                for q in range(8):
                    work.append((200, lambda i2=i2, q=q: combine(i2, q)))
                for c in range(4):
                    work += projT_thunks(p + 1, c, xin[i2], xpT_next)
                    work += xpn_thunks(p + 1, c, xin[i2], xpb_next)
            if p + 2 < NCH:
                work += gather_thunks(p + 2, xTb[(p + 2) % 2])
            if 2 <= p - 1:
                work += reduce_thunks(p - 1)
            wi = 0
            nwk = len(work)
            for ts in range(CH):
                t = p * CH + ts
                pin = (10 * t + 7) / 10000.0
                step(p, ts, xpb_cur)
                # prefill for the next step quad: r-part first (waits
                # sig-r only), drip, then z-part (waits sig-zb)
                pf = t + 1 < NSTEP and ts % 4 == 3
                if pf:
                    t2 = t + 1
                    prefill(t2, xpTb[(t2 // CH) % 2], 0, pin=pin)
                # drip this phase's background work into the chain gaps
                wtgt = 0 if ts < 6 else (nwk * (ts - 5) + CH - 7) // (CH - 6)
                if wi < wtgt:
                    with tc.tile_wait_until(pin):
                        while wi < wtgt:
                            work[wi][1]()
                            wi += 1
                if pf:
                    prefill(t2, xpTb[(t2 // CH) % 2], 1, pin=pin)
            # flush any un-dripped background work before the phase ends
            while wi < len(work):
                work[wi][1]()
                wi += 1
            if p < NCH:
                io = p % 2
                stv = st[:].rearrange("p (c s) -> p c s", c=KH)
                nc.gpsimd.dma_start(
                    out=snd_d[io].ap(),
                    in_=stv[:, :, (p * CH + 1) * BC:(p * CH + 1 + CH) * BC])
                nc.gpsimd.collective_compute(
                    "AllGather", mybir.AluOpType.bypass,
                    replica_groups=GROUPS,
                    ins=[snd_d[io].ap().opt()],
                    outs=[rcv_d[io].ap().opt()])
                nc.gpsimd.dma_start(out=rxS[p % 2][:],
                                    in_=rcv_d[io].ap()[0])
            xpb_cur = xpb_next

        for _, th in reduce_thunks(NPH - 1):
            th()
        nc.sync.dma_start(out=out_d[:], in_=pooled[:])

    nc.compile()
    return nc


def _prep(inputs):
    bf = ml_dtypes.bfloat16

    def packT(W, nk, negate_z=False, scale=1.0, rows=None, dt=None):
        Wf = np.asarray(W, np.float32).copy()
        if negate_z:
            Wf[H:2 * H] = -Wf[H:2 * H]
        Wf *= scale
        if rows is not None:
            Wf = Wf[rows[0]:rows[1]]
        WT = np.ascontiguousarray(Wf.T)
        K, G = WT.shape
        pad = np.zeros((nk * 128, G), np.float32)
        pad[:K] = WT
        return np.concatenate([pad[k * 128:(k + 1) * 128] for k in range(nk)],
                              axis=1).astype(dt if dt is not None else bf)

    texts = np.asarray(inputs["texts"])
    zidx = np.zeros((128, NBLK), np.int32)
    zemb = np.zeros((V, E), np.float32)
    sel = np.zeros((128, 64), bf)
    for pp in range(128):
        sel[pp, pp % 64] = 1.0
    in_maps = []
    for c in range(NCORE):
        l0 = c < DP
        q = c % DP
        act = [p < NCH for p in range(NPH)] if l0 else \
              [2 <= p for p in range(NPH)]
        Wih, Whh = (inputs["Wih0"], inputs["Whh0"]) if l0 else \
                   (inputs["Wih1"], inputs["Whh1"])
        bih, bhh = (inputs["bih0"], inputs["bhh0"]) if l0 else \
                   (inputs["bih1"], inputs["bhh1"])
        bih = np.asarray(bih, np.float32)
        bhh = np.asarray(bhh, np.float32)
        # transposed-proj bias row: r block, then NEGATED z block
        brow = np.zeros(G2, np.float32)
        brow[0:H] = bih[0:H] + bhh[0:H]
        brow[H:2 * H] = -(bih[H:2 * H] + bhh[H:2 * H])
        brz = np.zeros((128, NPH * G2), np.float32)
        b0n = np.zeros((128, NPH * 4), np.float32)
        ind4 = np.zeros((128, NPH * 64), np.float32)
        for p in range(NPH):
            if act[p]:
                brz[0, p * G2:(p + 1) * G2] = brow
                for mi in range(4):
                    b0n[:, p * 4 + mi] = bih[2 * H + 128 * mi:
                                             2 * H + 128 * (mi + 1)]
                    ind4[mi, p * 64 + 16 * mi:p * 64 + 16 * (mi + 1)] = 1.0
        bn4 = np.zeros((128, 128), np.float32)
        for j in range(4):
            bn4[j, :] = bhh[2 * H + 128 * j:2 * H + 128 * (j + 1)]
        idxc = np.ascontiguousarray(
            texts[:, q * BC:(q + 1) * BC].astype(np.int32)
            .reshape(NBLK, 128).T) if l0 else zidx
        in_maps.append({
            "emb": (np.ascontiguousarray(inputs["emb"], dtype=np.float32)
                    if l0 else zemb),
            "idx": idxc,
            "whh8": packT(Whh, KH, negate_z=True, scale=64.0, rows=(0, G2),
                          dt=ml_dtypes.float8_e3m4),
            "whhn": packT(Whh, KH, scale=64.0, rows=(G2, G3)),
            "wib": packT(Wih, KH, negate_z=True,
                         scale=(1.0 if l0 else 64.0)),
            "fl1": np.full((128, 1), 0.0 if l0 else 1.0, np.float32),
            "brz": brz.astype(bf),
            "b0n": b0n,
            "bn4": bn4.astype(bf),
            "ind4": ind4.astype(bf),
            "sel": sel,
        })
    return in_maps


def _postproc(results):
    s = np.zeros((128, KH * T), np.float32)
    for c in range(DP, NCORE):
        s += results[c]["out"]
    return s.reshape(128, KH, T).transpose(2, 1, 0).reshape(T, H) * (64.0 / B)


def kernel(**inputs):
    from concourse import bass_utils
    if "nc" not in _cache:
        _cache["nc"] = _build()
    nc = _cache["nc"]
    in_maps = _prep(inputs)
    res = bass_utils.run_bass_kernel_spmd(
        nc, in_maps, core_ids=list(range(NCORE)))
    pooled = _postproc([res.results[i] for i in range(NCORE)])
    fc_W = np.asarray(inputs["fc_W"], dtype=np.float32)
    fc_b = np.asarray(inputs["fc_b"], dtype=np.float32)
    return (pooled @ fc_W.T + fc_b).astype(np.float32)


if __name__ == "__main__":
    import time
    t0 = time.time()
    nc = _build()
    print("build+compile time:", round(time.time() - t0, 1), "s")


# revision 35
# speedup vs baseline: 1.2295x; 1.0189x over previous
"""2-layer GRU (T=512, B=64, E=300, H=512) on 8 NeuronCores.

Strategy v8: 4-way batch-parallel x 2-way layer-pipeline, rebuilt per-step
critical path:
  - z-gate weights/biases NEGATED at prep, so zb = sigmoid(az) = 1-z;
    combine h' = (h - zb*h) + zb*n runs its (h - zb*h) half in parallel
    with tanh; final add is a fused scalar_tensor_tensor that also
    rescales (see below).
  - r/z recurrent weights in fp8 E3M4 at x64 scale, n-gate in bf16 x64;
    the state is stored as h/64 (bf16), compensated exactly by the fused
    v*(1/64)+u combine and a x64 on the host reduce. wib (layer-1 input
    proj) is also x64 since it consumes h1/64.
  - input projections computed TRANSPOSED (xpT[token,gate]) via a
    unified per-core input buffer xin = rxS*flagL1 + xT (layer-1 cores
    get zero embeddings so their gather contributes zero), 4 uniform
    k-matmuls per 512-gate bank; per-step xr/xzb enter PSUM via k=64
    selector matmuls (one per m-tile per 4-step quad, split r/z around
    the drip to avoid PE queue head-blocking).
  - n-gate hidden bias via one k=4 matmul FIRST (start=True); n-gate xp
    stays in gate-major layout for the an vector-add.
  - background work (projections, gathers, combine) dripped as ~1-matmul
    thunks between steps; no dummy warm matmuls.
Host: final 4-way partial-sum over layer-1 cores, x64/B, tiny FC.
"""
import numpy as np
import ml_dtypes

T, B, E, H, V, L = 512, 64, 300, 512, 30000, 5
NCORE = 8
DP = 4                    # batch-parallel width
BC = B // DP              # 16 sequences per core
CH = 16                   # steps per chunk/phase
NCK = CH * BC // 128      # 128-token blocks per phase chunk
NCH = T // CH             # 16 data chunks
NPH = NCH + 2             # 18 phases (layer-1 lags two chunks)
G3 = 3 * H                # 1536
G2 = 2 * H                # 1024 (r+z gate block, transposed proj)
KH = H // 128             # 4 k-tiles (contraction)
M3 = G3 // 128            # 12 m-tiles (output gates)
CB = CH * BC              # 512 cols per chunk
SEG = (NPH * CH + 1) * BC  # cols per k-segment of state
NBLK = T * BC // 128      # 64 gather blocks of 128 tokens
EK = 3                    # E=300 -> 3 partial k-tiles (128,128,44)
NSTEP = NPH * CH

_cache = {}


def _build():
    from contextlib import ExitStack
    import concourse.bass as bass
    import concourse.mybir as mybir
    import concourse.tile as tile
    from concourse import bacc
    from concourse.masks import make_identity

    bf16, f32, i32 = mybir.dt.bfloat16, mybir.dt.float32, mybir.dt.int32
    AF = mybir.ActivationFunctionType
    OP = mybir.AluOpType

    nc = bacc.Bacc("TRN2", target_bir_lowering=False, debug=False,
                   num_devices=NCORE)
    emb_d = nc.dram_tensor("emb", [V, E], f32, kind="ExternalInput").ap()
    idx_d = nc.dram_tensor("idx", [128, NBLK], i32, kind="ExternalInput").ap()
    whh8_d = nc.dram_tensor("whh8", [128, KH * G2], mybir.dt.float8e3,
                            kind="ExternalInput").ap()
    whhn_d = nc.dram_tensor("whhn", [128, KH * 512], bf16,
                            kind="ExternalInput").ap()
    wib_d = nc.dram_tensor("wib", [128, KH * G3], bf16, kind="ExternalInput").ap()
    fl1_d = nc.dram_tensor("fl1", [128, 1], f32, kind="ExternalInput").ap()
    # transposed-proj bias row (r,zb blocks, phase-gated) on partition 0
    brz_d = nc.dram_tensor("brz", [128, NPH * G2], bf16, kind="ExternalInput").ap()
    # n-gate input-proj bias (bih_n), per phase, for the ACT bias-copy
    b0n_d = nc.dram_tensor("b0n", [128, NPH * 4], f32, kind="ExternalInput").ap()
    # n-gate hidden bias wrapped k=4: bn4[j, p] = bhh_n[128j + p]
    bn4_d = nc.dram_tensor("bn4", [128, 128], bf16, kind="ExternalInput").ap()
    # phase-gated k=4 indicator: ind4[j, p*64 + 16mi + b] = act[p]*(mi == j)
    ind4_d = nc.dram_tensor("ind4", [128, NPH * 64], bf16, kind="ExternalInput").ap()
    # EYE64: sel[p, c] = (p % 64 == c), for the k=64 prefill matmuls
    sel_d = nc.dram_tensor("sel", [128, 64], bf16, kind="ExternalInput").ap()
    out_d = nc.dram_tensor("out", [128, KH * T], f32, kind="ExternalOutput").ap()
    # collective bounce buffers (double-buffered across phases)
    snd_d = [nc.dram_tensor(f"snd{i}", [128, KH * CB], bf16) for i in range(2)]
    rcv_d = [nc.dram_tensor(f"rcv{i}", [2, 128, KH * CB], bf16) for i in range(2)]
    GROUPS = [[c, c + DP] for c in range(DP)]

    with tile.TileContext(nc) as tc, ExitStack() as ctx:
        wp = ctx.enter_context(tc.tile_pool(name="wp", bufs=1))
        sp = ctx.enter_context(tc.tile_pool(name="sp", bufs=1))
        xb = ctx.enter_context(tc.tile_pool(name="xb", bufs=2))
        rxp = ctx.enter_context(tc.tile_pool(name="rxp", bufs=2))
        tp = ctx.enter_context(tc.tile_pool(name="tp", bufs=3))
        # PSUM: gate banks r, z, n(+tmp) = 3; projT 2; old-n-xp 2; transpose 1
        pg = ctx.enter_context(tc.tile_pool(name="pg", bufs=1, space="PSUM"))
        pj = ctx.enter_context(tc.tile_pool(name="pj", bufs=2, space="PSUM"))
        px = ctx.enter_context(tc.tile_pool(name="px", bufs=2, space="PSUM"))
        pt = ctx.enter_context(tc.tile_pool(name="pt", bufs=1, space="PSUM"))

        def wtile(nm, shape, dt, src=None):
            t = wp.tile(shape, dt, name=nm, tag=nm)
            if src is not None:
                nc.sync.dma_start(out=t[:], in_=src[:])
            return t

        whh8 = wtile("whh8_t", [128, KH * G2], mybir.dt.float8e3, whh8_d)
        whhn = wtile("whhn_t", [128, KH * 512], bf16, whhn_d)
        wib = wtile("wib_t", [128, KH * G3], bf16, wib_d)
        fl1 = wtile("fl1_t", [128, 1], f32, fl1_d)
        brz = wtile("brz_t", [128, NPH * G2], bf16, brz_d)
        b0n = wtile("b0n_t", [128, NPH * 4], f32, b0n_d)
        bn4 = wtile("bn4_t", [128, 128], bf16, bn4_d)
        ind4 = wtile("ind4_t", [128, NPH * 64], bf16, ind4_d)
        sel = wtile("sel_t", [128, 64], bf16, sel_d)
        idx_t = wtile("idx_t", [128, NBLK], i32, idx_d)
        ident = wtile("ident", [128, 128], bf16)
        make_identity(nc, ident[:])
        ones = wtile("ones", [128, 128], bf16)
        nc.vector.memset(ones[0:1, :], 1.0)

        st = sp.tile([128, KH * SEG], bf16, name="st", tag="st")
        pooled = sp.tile([128, KH * T], f32, name="pooled", tag="pooled")
        for k in range(KH):
            nc.vector.memset(st[:, k * SEG:k * SEG + BC], 0.0)
        rxS = []
        xin = []
        for i in range(2):
            r = sp.tile([128, KH * CB], bf16, name=f"rxS{i}", tag=f"rxS{i}")
            nc.vector.memset(r[:], 0.0)
            rxS.append(r)
            xi = sp.tile([128, KH * CB], bf16, name=f"xin{i}", tag=f"xin{i}")
            nc.vector.memset(xi[:], 0.0)
            xin.append(xi)
        # gate PSUM banks, persistent layout:
        #   prz_r: [0:64] even-step r, [64:128] odd-step r
        #   prz_z: same for zb;  pn: [0:64] gn, [64:128] hm, [128:192] an
        prz_r = pg.tile([128, 512], f32, name="przr", tag="przr")
        prz_z = pg.tile([128, 512], f32, name="przz", tag="przz")
        pn = pg.tile([128, 512], f32, name="pn", tag="pn")

        def gather_thunks(c, xT):
            """indirect-gather + transpose chunk c tokens into xT buffer"""
            out = []
            for j in range(NCK):
                box = {}

                def t_a(j=j, box=box):
                    blk = NCK * c + j
                    xg = tp.tile([128, E], f32, name="xg", tag="xg")
                    nc.gpsimd.indirect_dma_start(
                        out=xg[:], out_offset=None, in_=emb_d[:],
                        in_offset=bass.IndirectOffsetOnAxis(
                            ap=idx_t[:, blk:blk + 1], axis=0))
                    xc = tp.tile([128, E], bf16, name="xc", tag="xc")
                    nc.vector.tensor_copy(out=xc[:], in_=xg[:])
                    box["xc"] = xc

                def t_b(j=j, box=box):
                    xc = box["xc"]
                    for e in range(EK):
                        ke = min(128, E - e * 128)
                        tps = pt.tile([128, 128], bf16, name="tps", tag="tps")
                        nc.tensor.transpose(out=tps[0:ke, :],
                                            in_=xc[:, e * 128:e * 128 + ke],
                                            identity=ident[:])
                        nc.vector.tensor_copy(
                            out=xT[0:ke, e * CB + j * 128:e * CB + (j + 1) * 128],
                            in_=tps[0:ke, :])
                out.append((300, t_a))
                out.append((650, t_b))
            return out

        def projT_thunks(p, c, xi, xpT):
            """transposed input projection for phase p, chunk c, r+z gates,
            as (cost_ns, fn) thunks, one MM each."""
            out = []
            for g in range(2):           # gate block: 0=r, 1=zb
                box = {}

                def t_bias(g=g, box=box):
                    pp = pj.tile([128, 512], f32, name="ppj", tag="ppj")
                    box["pp"] = pp
                    nc.tensor.matmul(
                        out=pp[:, 0:512],
                        lhsT=ones[0:1, 0:128],
                        rhs=brz[0:1, p * G2 + g * 512:p * G2 + (g + 1) * 512],
                        start=True, stop=False)
                out.append((340, t_bias))

                for k in range(KH):
                    def t_k(g=g, box=box, k=k):
                        nc.tensor.matmul(
                            out=box["pp"][:, 0:512],
                            lhsT=xi[:, k * CB + c * 128:k * CB + (c + 1) * 128],
                            rhs=wib[:, k * G3 + g * 512:k * G3 + (g + 1) * 512],
                            start=False, stop=(k == KH - 1))
                    out.append((430, t_k))

                def t_c1(g=g, box=box):
                    nc.scalar.activation(
                        out=xpT[:, c * G2 + g * 512:c * G2 + g * 512 + 256],
                        in_=box["pp"][:, 0:256], func=AF.Identity)

                def t_c2(g=g, box=box):
                    nc.scalar.activation(
                        out=xpT[:, c * G2 + g * 512 + 256:c * G2 + (g + 1) * 512],
                        in_=box["pp"][:, 256:512], func=AF.Identity)
                out.append((200, t_c1))
                out.append((200, t_c2))
            return out

        def xpn_thunks(p, i, xi, xpb):
            """input projection for the n-gate m-tile i (of 4)."""
            m = 8 + i
            box = {}
            out = []
            for k in range(KH):
                def t_k(k=k):
                    if k == 0:
                        box["pp"] = px.tile([128, CB], f32, name="xpp", tag="xpp")
                    nc.tensor.matmul(
                        out=box["pp"][:, 0:CB],
                        lhsT=wib[:, k * G3 + m * 128:k * G3 + (m + 1) * 128],
                        rhs=xi[:, k * CB:(k + 1) * CB],
                        start=(k == 0), stop=(k == KH - 1))
                out.append((430, t_k))

            for h0 in range(0, CB, 256):
                def t_c(h0=h0):
                    hw = min(256, CB - h0)
                    nc.vector.tensor_scalar_add(
                        out=xpb[:, i * CB + h0:i * CB + h0 + hw],
                        in0=box["pp"][:, h0:h0 + hw],
                        scalar1=b0n[:, p * 4 + i:p * 4 + i + 1])
                out.append((200, t_c))
            return out

        def prefill(t, xpT, gate, pin=None):
            """pre-add xr (gate=0) or xzb (gate=1) for steps (t..t+3) into
            the bank col-sets. One k=64 selector matmul per m-tile."""
            from contextlib import nullcontext
            ts = t % CH
            cc = ts // 8
            base = 16 * (ts % 8)        # 0 or 64 because t % 4 == 0
            bank = prz_r if gate == 0 else prz_z
            with tc.tile_wait_until(pin) if pin is not None else nullcontext():
                bv = bank[:, 0:256].rearrange("p (q c) -> p q c", q=4)
                for mi in range(4):
                    nc.tensor.matmul(
                        out=bv[:, :, 16 * mi:16 * mi + 16],
                        lhsT=xpT[base:base + 64,
                                 cc * G2 + gate * 512 + 128 * mi:
                                 cc * G2 + gate * 512 + 128 * (mi + 1)],
                        rhs=sel[base:base + 64, :]
                            .rearrange("p (q c) -> p q c", q=4),
                        start=True, stop=False, skip_group_check=True)

        def step(p, ts, xpb):
            t = p * CH + ts
            hc = 64 * (ts % 4)          # r/z bank col-set for this step
            pin = (10 * t) / 10000.0

            def mm_gates(bank, gate):
                for mi in range(4):
                    m = gate * 4 + mi
                    for k in range(KH):
                        nc.tensor.matmul(
                            out=bank[:, hc + mi * BC:hc + (mi + 1) * BC],
                            lhsT=whh8[:, k * G2 + m * 128:k * G2 + (m + 1) * 128],
                            rhs=st[:, k * SEG + t * BC:k * SEG + (t + 1) * BC],
                            start=False, stop=(k == KH - 1),
                            skip_group_check=True)

            # bias-MM first (fills the h'-wait), then r, n, z groups
            nc.tensor.matmul(
                out=pn[:, 0:64],
                lhsT=bn4[0:4, 0:128],
                rhs=ind4[0:4, p * 64:(p + 1) * 64],
                start=True, stop=False, skip_group_check=True)
            mm_gates(prz_r, 0)
            for mi in range(4):
                for k in range(KH):
                    nc.tensor.matmul(
                        out=pn[:, mi * BC:(mi + 1) * BC],
                        lhsT=whhn[:, k * 512 + mi * 128:k * 512 + (mi + 1) * 128],
                        rhs=st[:, k * SEG + t * BC:k * SEG + (t + 1) * BC],
                        start=False, stop=(k == KH - 1),
                        skip_group_check=True)
            mm_gates(prz_z, 1)

            def tmp(nm, dt=f32):
                return tp.tile([128, KH * BC], dt, name=nm, tag=nm)

            # ACT queue: sig(r), sig(zb), tanh -- in this order
            r = tmp("r", bf16)
            with tc.tile_wait_until(pin + 1 / 10000.0):
                nc.scalar.activation(out=r[:], in_=prz_r[:, hc:hc + 64],
                                     func=AF.Sigmoid)
            zb = tmp("zb", bf16)
            with tc.tile_wait_until(pin + 3 / 10000.0):
                nc.scalar.activation(out=zb[:], in_=prz_z[:, hc:hc + 64],
                                     func=AF.Sigmoid)

            # DVE queue: hm, an, t1, u, v, h'
            with tc.tile_wait_until(pin + 2 / 10000.0):
                nc.vector.tensor_tensor(out=pn[:, 64:128], in0=pn[:, 0:64],
                                        in1=r[:], op=OP.mult)
            xpv = xpb[:].rearrange("p (m s) -> p m s", m=4)
            sl = ts * BC
            with tc.tile_wait_until(pin + 2.4 / 10000.0):
                nc.vector.tensor_tensor(
                    out=pn[:, 128:192].rearrange("p (m b) -> p m b", b=BC),
                    in0=xpv[:, :, sl:sl + BC],
                    in1=pn[:, 64:128].rearrange("p (m b) -> p m b", b=BC),
                    op=OP.add)
            n = tmp("n", bf16)
            with tc.tile_wait_until(pin + 4 / 10000.0):
                nc.scalar.activation(out=n[:], in_=pn[:, 128:192], func=AF.Tanh)

            stv = st[:].rearrange("p (c s) -> p c s", c=KH)
            hview = stv[:, :, t * BC:(t + 1) * BC]
            t1 = tmp("t1")
            with tc.tile_wait_until(pin + 4.4 / 10000.0):
                nc.vector.tensor_tensor(
                    out=t1[:].rearrange("p (c b) -> p c b", c=KH),
                    in0=hview, in1=zb[:].rearrange("p (c b) -> p c b", c=KH),
                    op=OP.mult)
            u = tmp("u")
            nc.vector.tensor_tensor(
                out=u[:].rearrange("p (c b) -> p c b", c=KH),
                in0=hview,
                in1=t1[:].rearrange("p (c b) -> p c b", c=KH),
                op=OP.subtract)
            v = tmp("v")
            with tc.tile_wait_until(pin + 6 / 10000.0):
                nc.vector.tensor_tensor(out=v[:], in0=zb[:], in1=n[:],
                                        op=OP.mult)
            nc.vector.scalar_tensor_tensor(
                out=stv[:, :, (t + 1) * BC:(t + 2) * BC],
                in0=v[:].rearrange("p (c b) -> p c b", c=KH),
                scalar=1.0 / 64.0,
                in1=u[:].rearrange("p (c b) -> p c b", c=KH),
                op0=OP.mult, op1=OP.add)

        def reduce_thunks(q):
            """partial batch-sum of phase q's states into pooled (4 pieces)"""
            out = []
            for piece in range(CH // 8):
                def t_r(piece=piece):
                    ts0 = piece * 8
                    nc.vector.tensor_reduce(
                        out=pooled[:].rearrange("p (c t) -> p c t", c=KH)
                            [:, :, (q - 2) * CH + ts0:(q - 2) * CH + ts0 + 8],
                        in_=st[:].rearrange("p (c s b) -> p c s b", c=KH, b=BC)
                            [:, :, q * CH + 1 + ts0:q * CH + 1 + ts0 + 8, :],
                        axis=mybir.AxisListType.X, op=OP.add)
                out.append((600, t_r))
            return out

        # ---- main schedule ----
        xTb = []
        for i in range(2):
            xt = xb.tile([128, KH * CB], bf16, name=f"xT{i}", tag=f"xT{i}")
            nc.vector.memset(xt[:], 0.0)
            xTb.append(xt)
        xpTb = []
        for i in range(2):
            xpTb.append(sp.tile([128, NCK * G2], bf16, name=f"xpT{i}",
                                tag=f"xpT{i}"))
        for _, th in gather_thunks(0, xTb[0]):
            th()
        for _, th in gather_thunks(1, xTb[1]):
            th()
        xpb_cur = rxp.tile([128, 4 * CB], bf16, name="xpb", tag="xpb")

        def combine(i, q):
            nc.vector.scalar_tensor_tensor(
                out=xin[i][:, q * 256:(q + 1) * 256],
                in0=rxS[i][:, q * 256:(q + 1) * 256], scalar=fl1[:, 0:1],
                in1=xTb[i][:, q * 256:(q + 1) * 256], op0=OP.mult, op1=OP.add)

        for q in range(KH * CB // 256):
            combine(0, q)
        for c in range(NCK):
            for _, th in projT_thunks(0, c, xin[0], xpTb[0]):
                th()
        for i in range(4):
            for _, th in xpn_thunks(0, i, xin[0], xpb_cur):
                th()
        prefill(0, xpTb[0], 0)
        prefill(0, xpTb[0], 1)
        for p in range(NPH):
            xpb_next = (rxp.tile([128, 4 * CB], bf16, name="xpb", tag="xpb")
                        if p + 1 < NPH else None)
            xpT_next = xpTb[(p + 1) % 2]
            # background work queue for this phase, dripped ~2 thunks/step
            work = []
            if p + 1 < NPH:
                i2 = (p + 1) % 2
                for q in range(KH * CB // 256):
                    work.append((200, lambda i2=i2, q=q: combine(i2, q)))
                for c in range(NCK):
                    work += projT_thunks(p + 1, c, xin[i2], xpT_next)
                for c in range(4):
                    work += xpn_thunks(p + 1, c, xin[i2], xpb_next)
            if p + 2 < NCH:
                work += gather_thunks(p + 2, xTb[(p + 2) % 2])
            if 2 <= p - 1:
                work += reduce_thunks(p - 1)
            wi = 0
            nwk = len(work)
            for ts in range(CH):
                t = p * CH + ts
                pin = (10 * t + 7) / 10000.0
                step(p, ts, xpb_cur)
                # prefill for the next step quad: r-part first (waits
                # sig-r only), drip, then z-part (waits sig-zb)
                pf = t + 1 < NSTEP and ts % 4 == 3
                if pf:
                    t2 = t + 1
                    prefill(t2, xpTb[(t2 // CH) % 2], 0, pin=pin)
                # drip this phase's background work into the chain gaps
                wtgt = 0 if ts < 6 else (nwk * (ts - 5) + CH - 7) // (CH - 6)
                if wi < wtgt:
                    with tc.tile_wait_until(pin):
                        while wi < wtgt:
                            work[wi][1]()
                            wi += 1
                if pf:
                    prefill(t2, xpTb[(t2 // CH) % 2], 1, pin=pin)
            # flush any un-dripped background work before the phase ends
            while wi < len(work):
                work[wi][1]()
                wi += 1
            if p < NCH:
                io = p % 2
                stv = st[:].rearrange("p (c s) -> p c s", c=KH)
                nc.gpsimd.dma_start(
                    out=snd_d[io].ap(),
                    in_=stv[:, :, (p * CH + 1) * BC:(p * CH + 1 + CH) * BC])
                nc.gpsimd.collective_compute(
                    "AllGather", mybir.AluOpType.bypass,
                    replica_groups=GROUPS,
                    ins=[snd_d[io].ap().opt()],
                    outs=[rcv_d[io].ap().opt()])
                nc.gpsimd.dma_start(out=rxS[p % 2][:],
                                    in_=rcv_d[io].ap()[0])
            xpb_cur = xpb_next

        for _, th in reduce_thunks(NPH - 1):
            th()
        nc.sync.dma_start(out=out_d[:], in_=pooled[:])

    nc.compile()
    return nc


def _prep(inputs):
    bf = ml_dtypes.bfloat16

    def packT(W, nk, negate_z=False, scale=1.0, rows=None, dt=None):
        Wf = np.asarray(W, np.float32).copy()
        if negate_z:
            Wf[H:2 * H] = -Wf[H:2 * H]
        Wf *= scale
        if rows is not None:
            Wf = Wf[rows[0]:rows[1]]
        WT = np.ascontiguousarray(Wf.T)
        K, G = WT.shape
        pad = np.zeros((nk * 128, G), np.float32)
        pad[:K] = WT
        return np.concatenate([pad[k * 128:(k + 1) * 128] for k in range(nk)],
                              axis=1).astype(dt if dt is not None else bf)

    texts = np.asarray(inputs["texts"])
    zidx = np.zeros((128, NBLK), np.int32)
    zemb = np.zeros((V, E), np.float32)
    sel = np.zeros((128, 64), bf)
    for pp in range(128):
        sel[pp, pp % 64] = 1.0
    in_maps = []
    for c in range(NCORE):
        l0 = c < DP
        q = c % DP
        act = [p < NCH for p in range(NPH)] if l0 else \
              [2 <= p for p in range(NPH)]
        Wih, Whh = (inputs["Wih0"], inputs["Whh0"]) if l0 else \
                   (inputs["Wih1"], inputs["Whh1"])
        bih, bhh = (inputs["bih0"], inputs["bhh0"]) if l0 else \
                   (inputs["bih1"], inputs["bhh1"])
        bih = np.asarray(bih, np.float32)
        bhh = np.asarray(bhh, np.float32)
        # transposed-proj bias row: r block, then NEGATED z block
        brow = np.zeros(G2, np.float32)
        brow[0:H] = bih[0:H] + bhh[0:H]
        brow[H:2 * H] = -(bih[H:2 * H] + bhh[H:2 * H])
        brz = np.zeros((128, NPH * G2), np.float32)
        b0n = np.zeros((128, NPH * 4), np.float32)
        ind4 = np.zeros((128, NPH * 64), np.float32)
        for p in range(NPH):
            if act[p]:
                brz[0, p * G2:(p + 1) * G2] = brow
                for mi in range(4):
                    b0n[:, p * 4 + mi] = bih[2 * H + 128 * mi:
                                             2 * H + 128 * (mi + 1)]
                    ind4[mi, p * 64 + 16 * mi:p * 64 + 16 * (mi + 1)] = 1.0
        bn4 = np.zeros((128, 128), np.float32)
        for j in range(4):
            bn4[j, :] = bhh[2 * H + 128 * j:2 * H + 128 * (j + 1)]
        idxc = np.ascontiguousarray(
            texts[:, q * BC:(q + 1) * BC].astype(np.int32)
            .reshape(NBLK, 128).T) if l0 else zidx
        in_maps.append({
            "emb": (np.ascontiguousarray(inputs["emb"], dtype=np.float32)
                    if l0 else zemb),
            "idx": idxc,
            "whh8": packT(Whh, KH, negate_z=True, scale=64.0, rows=(0, G2),
                          dt=ml_dtypes.float8_e3m4),
            "whhn": packT(Whh, KH, scale=64.0, rows=(G2, G3)),
            "wib": packT(Wih, KH, negate_z=True,
                         scale=(1.0 if l0 else 64.0)),
            "fl1": np.full((128, 1), 0.0 if l0 else 1.0, np.float32),
            "brz": brz.astype(bf),
            "b0n": b0n,
            "bn4": bn4.astype(bf),
            "ind4": ind4.astype(bf),
            "sel": sel,
        })
    return in_maps


def _postproc(results):
    s = np.zeros((128, KH * T), np.float32)
    for c in range(DP, NCORE):
        s += results[c]["out"]
    return s.reshape(128, KH, T).transpose(2, 1, 0).reshape(T, H) * (64.0 / B)


def kernel(**inputs):
    from concourse import bass_utils
    if "nc" not in _cache:
        _cache["nc"] = _build()
    nc = _cache["nc"]
    in_maps = _prep(inputs)
    res = bass_utils.run_bass_kernel_spmd(
        nc, in_maps, core_ids=list(range(NCORE)))
    pooled = _postproc([res.results[i] for i in range(NCORE)])
    fc_W = np.asarray(inputs["fc_W"], dtype=np.float32)
    fc_b = np.asarray(inputs["fc_b"], dtype=np.float32)
    return (pooled @ fc_W.T + fc_b).astype(np.float32)


if __name__ == "__main__":
    import time
    t0 = time.time()
    nc = _build()
    print("build+compile time:", round(time.time() - t0, 1), "s")


# revision 36
# speedup vs baseline: 1.2332x; 1.0030x over previous
"""2-layer GRU (T=512, B=64, E=300, H=512) on 8 NeuronCores.

Strategy v8: 4-way batch-parallel x 2-way layer-pipeline, rebuilt per-step
critical path:
  - z-gate weights/biases NEGATED at prep, so zb = sigmoid(az) = 1-z;
    combine h' = (h - zb*h) + zb*n runs its (h - zb*h) half in parallel
    with tanh; final add is a fused scalar_tensor_tensor that also
    rescales (see below).
  - r/z recurrent weights in fp8 E3M4 at x64 scale, n-gate in bf16 x64;
    the state is stored as h/64 (bf16), compensated exactly by the fused
    v*(1/64)+u combine and a x64 on the host reduce. wib (layer-1 input
    proj) is also x64 since it consumes h1/64.
  - input projections computed TRANSPOSED (xpT[token,gate]) via a
    unified per-core input buffer xin = rxS*flagL1 + xT (layer-1 cores
    get zero embeddings so their gather contributes zero), 4 uniform
    k-matmuls per 512-gate bank; per-step xr/xzb enter PSUM via k=64
    selector matmuls (one per m-tile per 4-step quad, split r/z around
    the drip to avoid PE queue head-blocking).
  - n-gate hidden bias via one k=4 matmul FIRST (start=True); n-gate xp
    stays in gate-major layout for the an vector-add.
  - background work (projections, gathers, combine) dripped as ~1-matmul
    thunks between steps; no dummy warm matmuls.
Host: final 4-way partial-sum over layer-1 cores, x64/B, tiny FC.
"""
import numpy as np
import ml_dtypes

T, B, E, H, V, L = 512, 64, 300, 512, 30000, 5
NCORE = 8
DP = 4                    # batch-parallel width
BC = B // DP              # 16 sequences per core
CH = 16                   # steps per chunk/phase
NCK = CH * BC // 128      # 128-token blocks per phase chunk
NCH = T // CH             # 16 data chunks
NPH = NCH + 2             # 18 phases (layer-1 lags two chunks)
G3 = 3 * H                # 1536
G2 = 2 * H                # 1024 (r+z gate block, transposed proj)
KH = H // 128             # 4 k-tiles (contraction)
M3 = G3 // 128            # 12 m-tiles (output gates)
CB = CH * BC              # 512 cols per chunk
SEG = (NPH * CH + 1) * BC  # cols per k-segment of state
NBLK = T * BC // 128      # 64 gather blocks of 128 tokens
EK = 3                    # E=300 -> 3 partial k-tiles (128,128,44)
NSTEP = NPH * CH

_cache = {}


def _build():
    from contextlib import ExitStack
    import concourse.bass as bass
    import concourse.mybir as mybir
    import concourse.tile as tile
    from concourse import bacc
    from concourse.masks import make_identity

    bf16, f32, i32 = mybir.dt.bfloat16, mybir.dt.float32, mybir.dt.int32
    AF = mybir.ActivationFunctionType
    OP = mybir.AluOpType

    nc = bacc.Bacc("TRN2", target_bir_lowering=False, debug=False,
                   num_devices=NCORE)
    emb_d = nc.dram_tensor("emb", [V, E], f32, kind="ExternalInput").ap()
    idx_d = nc.dram_tensor("idx", [128, NBLK], i32, kind="ExternalInput").ap()
    whh8_d = nc.dram_tensor("whh8", [128, KH * G2], mybir.dt.float8e3,
                            kind="ExternalInput").ap()
    whhn_d = nc.dram_tensor("whhn", [128, KH * 512], bf16,
                            kind="ExternalInput").ap()
    wib_d = nc.dram_tensor("wib", [128, KH * G3], bf16, kind="ExternalInput").ap()
    fl1_d = nc.dram_tensor("fl1", [128, 1], f32, kind="ExternalInput").ap()
    # transposed-proj bias row (r,zb blocks, phase-gated) on partition 0
    brz_d = nc.dram_tensor("brz", [128, NPH * G2], bf16, kind="ExternalInput").ap()
    # n-gate input-proj bias (bih_n), per phase, for the ACT bias-copy
    b0n_d = nc.dram_tensor("b0n", [128, NPH * 4], f32, kind="ExternalInput").ap()
    # n-gate hidden bias wrapped k=4: bn4[j, p] = bhh_n[128j + p]
    bn4_d = nc.dram_tensor("bn4", [128, 128], bf16, kind="ExternalInput").ap()
    # phase-gated k=4 indicator: ind4[j, p*64 + 16mi + b] = act[p]*(mi == j)
    ind4_d = nc.dram_tensor("ind4", [128, NPH * 64], bf16, kind="ExternalInput").ap()
    # EYE64: sel[p, c] = (p % 64 == c), for the k=64 prefill matmuls
    sel_d = nc.dram_tensor("sel", [128, 64], bf16, kind="ExternalInput").ap()
    out_d = nc.dram_tensor("out", [128, KH * T], f32, kind="ExternalOutput").ap()
    # collective bounce buffers (double-buffered across phases)
    snd_d = [nc.dram_tensor(f"snd{i}", [128, KH * CB], bf16) for i in range(2)]
    rcv_d = [nc.dram_tensor(f"rcv{i}", [2, 128, KH * CB], bf16) for i in range(2)]
    GROUPS = [[c, c + DP] for c in range(DP)]

    with tile.TileContext(nc) as tc, ExitStack() as ctx:
        wp = ctx.enter_context(tc.tile_pool(name="wp", bufs=1))
        sp = ctx.enter_context(tc.tile_pool(name="sp", bufs=1))
        xb = ctx.enter_context(tc.tile_pool(name="xb", bufs=2))
        rxp = ctx.enter_context(tc.tile_pool(name="rxp", bufs=2))
        tp = ctx.enter_context(tc.tile_pool(name="tp", bufs=3))
        # PSUM: gate banks r, z, n(+tmp) = 3; projT 2; old-n-xp 2; transpose 1
        pg = ctx.enter_context(tc.tile_pool(name="pg", bufs=1, space="PSUM"))
        pj = ctx.enter_context(tc.tile_pool(name="pj", bufs=2, space="PSUM"))
        px = ctx.enter_context(tc.tile_pool(name="px", bufs=2, space="PSUM"))
        pt = ctx.enter_context(tc.tile_pool(name="pt", bufs=1, space="PSUM"))

        def wtile(nm, shape, dt, src=None):
            t = wp.tile(shape, dt, name=nm, tag=nm)
            if src is not None:
                nc.sync.dma_start(out=t[:], in_=src[:])
            return t

        whh8 = wtile("whh8_t", [128, KH * G2], mybir.dt.float8e3, whh8_d)
        whhn = wtile("whhn_t", [128, KH * 512], bf16, whhn_d)
        wib = wtile("wib_t", [128, KH * G3], bf16, wib_d)
        fl1 = wtile("fl1_t", [128, 1], f32, fl1_d)
        brz = wtile("brz_t", [128, NPH * G2], bf16, brz_d)
        b0n = wtile("b0n_t", [128, NPH * 4], f32, b0n_d)
        bn4 = wtile("bn4_t", [128, 128], bf16, bn4_d)
        ind4 = wtile("ind4_t", [128, NPH * 64], bf16, ind4_d)
        sel = wtile("sel_t", [128, 64], bf16, sel_d)
        idx_t = wtile("idx_t", [128, NBLK], i32, idx_d)
        ident = wtile("ident", [128, 128], bf16)
        make_identity(nc, ident[:])
        ones = wtile("ones", [128, 128], bf16)
        nc.vector.memset(ones[0:1, :], 1.0)

        st = sp.tile([128, KH * SEG], bf16, name="st", tag="st")
        pooled = sp.tile([128, KH * T], f32, name="pooled", tag="pooled")
        for k in range(KH):
            nc.vector.memset(st[:, k * SEG:k * SEG + BC], 0.0)
        rxS = []
        xin = []
        for i in range(2):
            r = sp.tile([128, KH * CB], bf16, name=f"rxS{i}", tag=f"rxS{i}")
            nc.vector.memset(r[:], 0.0)
            rxS.append(r)
            xi = sp.tile([128, KH * CB], bf16, name=f"xin{i}", tag=f"xin{i}")
            nc.vector.memset(xi[:], 0.0)
            xin.append(xi)
        # gate PSUM banks, persistent layout:
        #   prz_r: [0:64] even-step r, [64:128] odd-step r
        #   prz_z: same for zb;  pn: [0:64] gn, [64:128] hm, [128:192] an
        prz_r = pg.tile([128, 512], f32, name="przr", tag="przr")
        prz_z = pg.tile([128, 512], f32, name="przz", tag="przz")
        pn = pg.tile([128, 512], f32, name="pn", tag="pn")

        def gather_thunks(c, xT):
            """indirect-gather + transpose chunk c tokens into xT buffer"""
            out = []
            for j in range(NCK):
                box = {}

                def t_a(j=j, box=box):
                    blk = NCK * c + j
                    xg = tp.tile([128, E], f32, name="xg", tag="xg")
                    nc.gpsimd.indirect_dma_start(
                        out=xg[:], out_offset=None, in_=emb_d[:],
                        in_offset=bass.IndirectOffsetOnAxis(
                            ap=idx_t[:, blk:blk + 1], axis=0))
                    xc = tp.tile([128, E], bf16, name="xc", tag="xc")
                    nc.vector.tensor_copy(out=xc[:], in_=xg[:])
                    box["xc"] = xc

                def t_b(j=j, box=box):
                    xc = box["xc"]
                    for e in range(EK):
                        ke = min(128, E - e * 128)
                        tps = pt.tile([128, 128], bf16, name="tps", tag="tps")
                        nc.tensor.transpose(out=tps[0:ke, :],
                                            in_=xc[:, e * 128:e * 128 + ke],
                                            identity=ident[:])
                        nc.vector.tensor_copy(
                            out=xT[0:ke, e * CB + j * 128:e * CB + (j + 1) * 128],
                            in_=tps[0:ke, :])
                out.append((300, t_a))
                out.append((650, t_b))
            return out

        def projT_thunks(p, c, xi, xpT):
            """transposed input projection for phase p, chunk c, r+z gates,
            as (cost_ns, fn) thunks, one MM each."""
            out = []
            for g in range(2):           # gate block: 0=r, 1=zb
                box = {}

                def t_bias(g=g, box=box):
                    pp = pj.tile([128, 512], f32, name="ppj", tag="ppj")
                    box["pp"] = pp
                    nc.tensor.matmul(
                        out=pp[:, 0:512],
                        lhsT=ones[0:1, 0:128],
                        rhs=brz[0:1, p * G2 + g * 512:p * G2 + (g + 1) * 512],
                        start=True, stop=False)
                out.append((340, t_bias))

                for k in range(KH):
                    def t_k(g=g, box=box, k=k):
                        nc.tensor.matmul(
                            out=box["pp"][:, 0:512],
                            lhsT=xi[:, k * CB + c * 128:k * CB + (c + 1) * 128],
                            rhs=wib[:, k * G3 + g * 512:k * G3 + (g + 1) * 512],
                            start=False, stop=(k == KH - 1))
                    out.append((430, t_k))

                def t_c1(g=g, box=box):
                    nc.scalar.activation(
                        out=xpT[:, c * G2 + g * 512:c * G2 + g * 512 + 256],
                        in_=box["pp"][:, 0:256], func=AF.Identity)

                def t_c2(g=g, box=box):
                    nc.scalar.activation(
                        out=xpT[:, c * G2 + g * 512 + 256:c * G2 + (g + 1) * 512],
                        in_=box["pp"][:, 256:512], func=AF.Identity)
                out.append((200, t_c1))
                out.append((200, t_c2))
            return out

        def xpn_thunks(p, i, xi, xpb):
            """input projection for the n-gate m-tile i (of 4)."""
            m = 8 + i
            box = {}
            out = []
            for k in range(KH):
                def t_k(k=k):
                    if k == 0:
                        box["pp"] = px.tile([128, CB], f32, name="xpp", tag="xpp")
                    nc.tensor.matmul(
                        out=box["pp"][:, 0:CB],
                        lhsT=wib[:, k * G3 + m * 128:k * G3 + (m + 1) * 128],
                        rhs=xi[:, k * CB:(k + 1) * CB],
                        start=(k == 0), stop=(k == KH - 1))
                out.append((430, t_k))

            for h0 in range(0, CB, 256):
                def t_c(h0=h0):
                    hw = min(256, CB - h0)
                    nc.vector.tensor_scalar_add(
                        out=xpb[:, i * CB + h0:i * CB + h0 + hw],
                        in0=box["pp"][:, h0:h0 + hw],
                        scalar1=b0n[:, p * 4 + i:p * 4 + i + 1])
                out.append((200, t_c))
            return out

        def prefill(t, xpT, gate, pin=None):
            """pre-add xr (gate=0) or xzb (gate=1) for steps (t..t+3) into
            the bank col-sets. One k=64 selector matmul per m-tile."""
            from contextlib import nullcontext
            ts = t % CH
            cc = ts // 8
            base = 16 * (ts % 8)        # 0 or 64 because t % 4 == 0
            bank = prz_r if gate == 0 else prz_z
            with tc.tile_wait_until(pin) if pin is not None else nullcontext():
                bv = bank[:, 0:256].rearrange("p (q c) -> p q c", q=4)
                for mi in range(4):
                    nc.tensor.matmul(
                        out=bv[:, :, 16 * mi:16 * mi + 16],
                        lhsT=xpT[base:base + 64,
                                 cc * G2 + gate * 512 + 128 * mi:
                                 cc * G2 + gate * 512 + 128 * (mi + 1)],
                        rhs=sel[base:base + 64, :]
                            .rearrange("p (q c) -> p q c", q=4),
                        start=True, stop=False, skip_group_check=True)

        def step(p, ts, xpb):
            t = p * CH + ts
            hc = 64 * (ts % 4)          # r/z bank col-set for this step
            pin = (10 * t) / 10000.0

            def mm_gates(bank, gate):
                for mi in range(4):
                    m = gate * 4 + mi
                    for k in range(KH):
                        nc.tensor.matmul(
                            out=bank[:, hc + mi * BC:hc + (mi + 1) * BC],
                            lhsT=whh8[:, k * G2 + m * 128:k * G2 + (m + 1) * 128],
                            rhs=st[:, k * SEG + t * BC:k * SEG + (t + 1) * BC],
                            start=False, stop=(k == KH - 1),
                            skip_group_check=True)

            # bias-MM first (fills the h'-wait), then r, n, z groups
            nc.tensor.matmul(
                out=pn[:, 0:64],
                lhsT=bn4[0:4, 0:128],
                rhs=ind4[0:4, p * 64:(p + 1) * 64],
                start=True, stop=False, skip_group_check=True)
            mm_gates(prz_r, 0)
            for mi in range(4):
                for k in range(KH):
                    nc.tensor.matmul(
                        out=pn[:, mi * BC:(mi + 1) * BC],
                        lhsT=whhn[:, k * 512 + mi * 128:k * 512 + (mi + 1) * 128],
                        rhs=st[:, k * SEG + t * BC:k * SEG + (t + 1) * BC],
                        start=False, stop=(k == KH - 1),
                        skip_group_check=True)
            mm_gates(prz_z, 1)

            def tmp(nm, dt=f32):
                return tp.tile([128, KH * BC], dt, name=nm, tag=nm)

            # ACT queue: sig(r), sig(zb), tanh -- in this order
            r = tmp("r", bf16)
            with tc.tile_wait_until(pin + 1 / 10000.0):
                nc.scalar.activation(out=r[:], in_=prz_r[:, hc:hc + 64],
                                     func=AF.Sigmoid)
            zb = tmp("zb", bf16)
            with tc.tile_wait_until(pin + 3 / 10000.0):
                nc.scalar.activation(out=zb[:], in_=prz_z[:, hc:hc + 64],
                                     func=AF.Sigmoid)

            # DVE queue: hm, an, t1, u, v, h'
            with tc.tile_wait_until(pin + 2 / 10000.0):
                nc.vector.tensor_tensor(out=pn[:, 64:128], in0=pn[:, 0:64],
                                        in1=r[:], op=OP.mult)
            xpv = xpb[:].rearrange("p (m s) -> p m s", m=4)
            sl = ts * BC
            with tc.tile_wait_until(pin + 2.4 / 10000.0):
                nc.vector.tensor_tensor(
                    out=pn[:, 128:192].rearrange("p (m b) -> p m b", b=BC),
                    in0=xpv[:, :, sl:sl + BC],
                    in1=pn[:, 64:128].rearrange("p (m b) -> p m b", b=BC),
                    op=OP.add)
            n = tmp("n", bf16)
            with tc.tile_wait_until(pin + 4 / 10000.0):
                nc.scalar.activation(out=n[:], in_=pn[:, 128:192], func=AF.Tanh)

            stv = st[:].rearrange("p (c s) -> p c s", c=KH)
            hview = stv[:, :, t * BC:(t + 1) * BC]
            t1 = tmp("t1")
            with tc.tile_wait_until(pin + 4.4 / 10000.0):
                nc.vector.tensor_tensor(
                    out=t1[:].rearrange("p (c b) -> p c b", c=KH),
                    in0=hview, in1=zb[:].rearrange("p (c b) -> p c b", c=KH),
                    op=OP.mult)
            u = tmp("u")
            nc.vector.tensor_tensor(
                out=u[:].rearrange("p (c b) -> p c b", c=KH),
                in0=hview,
                in1=t1[:].rearrange("p (c b) -> p c b", c=KH),
                op=OP.subtract)
            v = tmp("v")
            with tc.tile_wait_until(pin + 6 / 10000.0):
                nc.vector.tensor_tensor(out=v[:], in0=zb[:], in1=n[:],
                                        op=OP.mult)
            nc.vector.scalar_tensor_tensor(
                out=stv[:, :, (t + 1) * BC:(t + 2) * BC],
                in0=v[:].rearrange("p (c b) -> p c b", c=KH),
                scalar=1.0 / 64.0,
                in1=u[:].rearrange("p (c b) -> p c b", c=KH),
                op0=OP.mult, op1=OP.add)

        def reduce_thunks(q):
            """partial batch-sum of phase q's states into pooled (4 pieces)"""
            out = []
            for piece in range(CH // 8):
                def t_r(piece=piece):
                    ts0 = piece * 8
                    nc.vector.tensor_reduce(
                        out=pooled[:].rearrange("p (c t) -> p c t", c=KH)
                            [:, :, (q - 2) * CH + ts0:(q - 2) * CH + ts0 + 8],
                        in_=st[:].rearrange("p (c s b) -> p c s b", c=KH, b=BC)
                            [:, :, q * CH + 1 + ts0:q * CH + 1 + ts0 + 8, :],
                        axis=mybir.AxisListType.X, op=OP.add)
                out.append((600, t_r))
            return out

        # ---- main schedule ----
        xTb = []
        for i in range(2):
            xt = xb.tile([128, KH * CB], bf16, name=f"xT{i}", tag=f"xT{i}")
            nc.vector.memset(xt[:], 0.0)
            xTb.append(xt)
        xpTb = []
        for i in range(2):
            xpTb.append(sp.tile([128, NCK * G2], bf16, name=f"xpT{i}",
                                tag=f"xpT{i}"))
        for _, th in gather_thunks(0, xTb[0]):
            th()
        for _, th in gather_thunks(1, xTb[1]):
            th()
        xpb_cur = rxp.tile([128, 4 * CB], bf16, name="xpb", tag="xpb")

        def combine(i, q):
            nc.vector.scalar_tensor_tensor(
                out=xin[i][:, q * 256:(q + 1) * 256],
                in0=rxS[i][:, q * 256:(q + 1) * 256], scalar=fl1[:, 0:1],
                in1=xTb[i][:, q * 256:(q + 1) * 256], op0=OP.mult, op1=OP.add)

        for q in range(KH * CB // 256):
            combine(0, q)
        for c in range(NCK):
            for _, th in projT_thunks(0, c, xin[0], xpTb[0]):
                th()
        for i in range(4):
            for _, th in xpn_thunks(0, i, xin[0], xpb_cur):
                th()
        prefill(0, xpTb[0], 0)
        prefill(0, xpTb[0], 1)
        for p in range(NPH):
            xpb_next = (rxp.tile([128, 4 * CB], bf16, name="xpb", tag="xpb")
                        if p + 1 < NPH else None)
            xpT_next = xpTb[(p + 1) % 2]
            # background work queue for this phase, dripped ~2 thunks/step
            work = []
            if p + 1 < NPH:
                i2 = (p + 1) % 2
                for q in range(KH * CB // 256):
                    work.append((200, lambda i2=i2, q=q: combine(i2, q)))
                for c in range(NCK):
                    work += projT_thunks(p + 1, c, xin[i2], xpT_next)
                for c in range(4):
                    work += xpn_thunks(p + 1, c, xin[i2], xpb_next)
            if p + 2 < NCH:
                work += gather_thunks(p + 2, xTb[(p + 2) % 2])
            if 2 <= p - 1:
                work += reduce_thunks(p - 1)
            wi = 0
            nwk = len(work)
            for ts in range(CH):
                t = p * CH + ts
                pin = (10 * t + 7) / 10000.0
                step(p, ts, xpb_cur)
                # prefill for the next step quad: r-part first (waits
                # sig-r only), drip, then z-part (waits sig-zb)
                pf = t + 1 < NSTEP and ts % 4 == 3
                if pf:
                    t2 = t + 1
                    prefill(t2, xpTb[(t2 // CH) % 2], 0, pin=pin)
                # drip this phase's background work into the chain gaps
                wtgt = 0 if ts < 4 else (nwk * (ts - 3) + CH - 5) // (CH - 4)
                if wi < wtgt:
                    with tc.tile_wait_until(pin):
                        while wi < wtgt:
                            work[wi][1]()
                            wi += 1
                if pf:
                    prefill(t2, xpTb[(t2 // CH) % 2], 1, pin=pin)
            # flush any un-dripped background work before the phase ends
            while wi < len(work):
                work[wi][1]()
                wi += 1
            if p < NCH:
                io = p % 2
                stv = st[:].rearrange("p (c s) -> p c s", c=KH)
                nc.gpsimd.dma_start(
                    out=snd_d[io].ap(),
                    in_=stv[:, :, (p * CH + 1) * BC:(p * CH + 1 + CH) * BC])
                nc.gpsimd.collective_compute(
                    "AllGather", mybir.AluOpType.bypass,
                    replica_groups=GROUPS,
                    ins=[snd_d[io].ap().opt()],
                    outs=[rcv_d[io].ap().opt()])
                nc.gpsimd.dma_start(out=rxS[p % 2][:],
                                    in_=rcv_d[io].ap()[0])
            xpb_cur = xpb_next

        for _, th in reduce_thunks(NPH - 1):
            th()
        nc.sync.dma_start(out=out_d[:], in_=pooled[:])

    nc.compile()
    return nc


def _prep(inputs):
    bf = ml_dtypes.bfloat16

    def packT(W, nk, negate_z=False, scale=1.0, rows=None, dt=None):
        Wf = np.asarray(W, np.float32).copy()
        if negate_z:
            Wf[H:2 * H] = -Wf[H:2 * H]
        Wf *= scale
        if rows is not None:
            Wf = Wf[rows[0]:rows[1]]
        WT = np.ascontiguousarray(Wf.T)
        K, G = WT.shape
        pad = np.zeros((nk * 128, G), np.float32)
        pad[:K] = WT
        return np.concatenate([pad[k * 128:(k + 1) * 128] for k in range(nk)],
                              axis=1).astype(dt if dt is not None else bf)

    texts = np.asarray(inputs["texts"])
    zidx = np.zeros((128, NBLK), np.int32)
    zemb = np.zeros((V, E), np.float32)
    sel = np.zeros((128, 64), bf)
    for pp in range(128):
        sel[pp, pp % 64] = 1.0
    in_maps = []
    for c in range(NCORE):
        l0 = c < DP
        q = c % DP
        act = [p < NCH for p in range(NPH)] if l0 else \
              [2 <= p for p in range(NPH)]
        Wih, Whh = (inputs["Wih0"], inputs["Whh0"]) if l0 else \
                   (inputs["Wih1"], inputs["Whh1"])
        bih, bhh = (inputs["bih0"], inputs["bhh0"]) if l0 else \
                   (inputs["bih1"], inputs["bhh1"])
        bih = np.asarray(bih, np.float32)
        bhh = np.asarray(bhh, np.float32)
        # transposed-proj bias row: r block, then NEGATED z block
        brow = np.zeros(G2, np.float32)
        brow[0:H] = bih[0:H] + bhh[0:H]
        brow[H:2 * H] = -(bih[H:2 * H] + bhh[H:2 * H])
        brz = np.zeros((128, NPH * G2), np.float32)
        b0n = np.zeros((128, NPH * 4), np.float32)
        ind4 = np.zeros((128, NPH * 64), np.float32)
        for p in range(NPH):
            if act[p]:
                brz[0, p * G2:(p + 1) * G2] = brow
                for mi in range(4):
                    b0n[:, p * 4 + mi] = bih[2 * H + 128 * mi:
                                             2 * H + 128 * (mi + 1)]
                    ind4[mi, p * 64 + 16 * mi:p * 64 + 16 * (mi + 1)] = 1.0
        bn4 = np.zeros((128, 128), np.float32)
        for j in range(4):
            bn4[j, :] = bhh[2 * H + 128 * j:2 * H + 128 * (j + 1)]
        idxc = np.ascontiguousarray(
            texts[:, q * BC:(q + 1) * BC].astype(np.int32)
            .reshape(NBLK, 128).T) if l0 else zidx
        in_maps.append({
            "emb": (np.ascontiguousarray(inputs["emb"], dtype=np.float32)
                    if l0 else zemb),
            "idx": idxc,
            "whh8": packT(Whh, KH, negate_z=True, scale=64.0, rows=(0, G2),
                          dt=ml_dtypes.float8_e3m4),
            "whhn": packT(Whh, KH, scale=64.0, rows=(G2, G3)),
            "wib": packT(Wih, KH, negate_z=True,
                         scale=(1.0 if l0 else 64.0)),
            "fl1": np.full((128, 1), 0.0 if l0 else 1.0, np.float32),
            "brz": brz.astype(bf),
            "b0n": b0n,
            "bn4": bn4.astype(bf),
            "ind4": ind4.astype(bf),
            "sel": sel,
        })
    return in_maps


def _postproc(results):
    s = np.zeros((128, KH * T), np.float32)
    for c in range(DP, NCORE):
        s += results[c]["out"]
    return s.reshape(128, KH, T).transpose(2, 1, 0).reshape(T, H) * (64.0 / B)


def kernel(**inputs):
    from concourse import bass_utils
    if "nc" not in _cache:
        _cache["nc"] = _build()
    nc = _cache["nc"]
    in_maps = _prep(inputs)
    res = bass_utils.run_bass_kernel_spmd(
        nc, in_maps, core_ids=list(range(NCORE)))
    pooled = _postproc([res.results[i] for i in range(NCORE)])
    fc_W = np.asarray(inputs["fc_W"], dtype=np.float32)
    fc_b = np.asarray(inputs["fc_b"], dtype=np.float32)
    return (pooled @ fc_W.T + fc_b).astype(np.float32)


if __name__ == "__main__":
    import time
    t0 = time.time()
    nc = _build()
    print("build+compile time:", round(time.time() - t0, 1), "s")
